# revision 1
# baseline (speedup 1.0000x reference)
"""AnchorFreeLoss on 8 TRN2 NeuronCores.

Strategy (data-parallel over batch, 2 images/core):
- Heatmap target: logG[pix, m] = -dist^2/(2*sigma_m^2) is an affine function of
  q(pix) = [x^2+y^2, x, y, 1]  ->  one PE matmul per 128-pixel chunk with
  per-object coefficient matrix W (built on device from bboxes).
  Output lands in PSUM [128 pix, (2 img, 64 m)]; DVE tensor_reduce(max) along
  the free m axis gives log-heatmap per pixel. Since sigma = r/2 exactly, the
  reference's dist<=2r cutoff equals logG >= -8 (constant!), applied post-max.
- Box/class losses only receive nonzero contributions at object-center cells,
  so pred_boxes/pred_classes are gathered sparsely (indirect DMA over
  host-transposed [B*H*W, C] tables); 77MB of dense reads avoided.
- Duplicate-cell collisions are deduplicated on device with a [128,128]
  is_equal matrix (last valid object wins, matching XLA scatter semantics).
- Partial sums are AllGathered (cheaper than AllReduce) and reduced on-device.
"""

import sys
from contextlib import ExitStack

import numpy as np

if "/opt/trn_rl_repo" not in sys.path:
    sys.path.insert(0, "/opt/trn_rl_repo")

from concourse import bass, bass_isa, mybir
from concourse.bass_utils import run_bass_kernel_spmd

F32 = mybir.dt.float32
I32 = mybir.dt.int32
ALU = mybir.AluOpType
ACT = mybir.ActivationFunctionType

B, M, H, W = 16, 64, 160, 160
NC = 8
BPC = B // NC          # 2 images per core
PIX = H * W            # 25600
NP2 = 2 * M            # 128 = objects of both images on one partition axis
NCLS = 43
EPS = 1e-7
LN4 = 1.3862943611198906
THR = -8.0             # log-domain cutoff (= dist <= 2r since sigma = r/2)
NGRP = PIX // 512      # 50 groups of 4 chunks x 128 pixels
NBANK = 7              # psum pipeline depth


class _DrainAfter:
    """Wrap an engine so every compute op is followed by a pipeline drain().

    Raw-bass DVE/Act programs have no hazard interlock between dependent
    back-to-back ops on the same engine; the interpreter's race detector
    confirms a drain (or semaphore) is required before a read-after-write.
    """

    def __init__(self, eng):
        self._e = eng

    def __getattr__(self, name):
        f = getattr(self._e, name)
        if name in ("wait_ge", "sem_inc", "drain", "then_inc"):
            return f
        def g(*a, **k):
            r = f(*a, **k)
            self._e.drain()
            return r
        return g


def _build(debug=False):
    nc = bass.Bass()

    hm_d = nc.declare_dram_parameter("hm", [BPC, PIX], F32, isOutput=False)
    pb_d = nc.declare_dram_parameter("pbt", [BPC * PIX, 4], F32, isOutput=False)
    pc_d = nc.declare_dram_parameter("pct", [BPC * PIX, NCLS], F32, isOutput=False)
    bb_d = nc.declare_dram_parameter("bb", [BPC, M, 4], F32, isOutput=False)
    lab_d = nc.declare_dram_parameter("lab", [BPC, M], I32, isOutput=False)
    qg_d = nc.declare_dram_parameter("qgrid", [4, PIX], F32, isOutput=False)
    ut_d = nc.declare_dram_parameter("utri", [128, 128], F32, isOutput=False)
    cv_d = nc.declare_dram_parameter("cvec", [128, 8], F32, isOutput=False)
    ch_d = nc.declare_dram_parameter("chm", [128, NCLS], F32, isOutput=False)
    out_d = nc.declare_dram_parameter("out", [1, 1], F32, isOutput=True)
    dbg = {}
    if debug:
        for nm, shp, dt in [("d_partials", [128, 8], F32), ("d_pvec", [1, 8], F32),
                            ("d_agv", [8, 8], F32), ("d_sc", [128, 40], F32)]:
            dbg[nm] = nc.declare_dram_parameter(nm, shp, dt, isOutput=True)

    cc_in = nc.dram_tensor("cc_in", [1, 8], F32)
    cc_out = nc.dram_tensor("cc_out", [8, 8], F32, addr_space="Shared")
    dbc = nc.dram_tensor("dbc", [2, 128], F32)

    core_ids = list(range(NC))

    es = ExitStack()
    dma_in = es.enter_context(nc.semaphore("dma_in"))
    dma2 = es.enter_context(nc.semaphore("dma2"))
    d4 = es.enter_context(nc.semaphore("d4"))
    d5 = es.enter_context(nc.semaphore("d5"))
    d6 = es.enter_context(nc.semaphore("d6"))
    va = es.enter_context(nc.semaphore("va"))
    av = es.enter_context(nc.semaphore("av"))
    wrdy = es.enter_context(nc.semaphore("wrdy"))
    tr_s = es.enter_context(nc.semaphore("tr_s"))
    pe_s = es.enter_context(nc.semaphore("pe_s"))
    dv_s = es.enter_context(nc.semaphore("dv_s"))
    cell_s = es.enter_context(nc.semaphore("cell_s"))
    g_s = es.enter_context(nc.semaphore("g_s"))
    pt_s = es.enter_context(nc.semaphore("pt_s"))
    pp_s = es.enter_context(nc.semaphore("pp_s"))
    pv_s = es.enter_context(nc.semaphore("pv_s"))
    cc_s = es.enter_context(nc.semaphore("cc_s"))
    par_s = es.enter_context(nc.semaphore("par_s"))
    fin_s = es.enter_context(nc.semaphore("fin_s"))
    sQ = es.enter_context(nc.sbuf_tensor("sQ", [4, PIX], F32))
    sU = es.enter_context(nc.sbuf_tensor("sU", [128, 128], F32))
    cvec = es.enter_context(nc.sbuf_tensor("cvec_s", [128, 8], F32))
    chm = es.enter_context(nc.sbuf_tensor("chm_s", [128, NCLS], F32))
    sbb = es.enter_context(nc.sbuf_tensor("sbb", [128, 4], F32))
    slab = es.enter_context(nc.sbuf_tensor("slab", [128, 1], I32))
    hmP = es.enter_context(nc.sbuf_tensor("hmP", [128, 400], F32))
    hmL = es.enter_context(nc.sbuf_tensor("hmL", [128, 400], F32))
    W5 = es.enter_context(nc.sbuf_tensor("W5", [128, 32], F32))
    tmp = es.enter_context(nc.sbuf_tensor("tmp", [32, 128], F32))
    cbc = es.enter_context(nc.sbuf_tensor("cbc", [128, 128], F32))
    kbc = es.enter_context(nc.sbuf_tensor("kbc", [128, 128], F32))
    eqt = es.enter_context(nc.sbuf_tensor("eqt", [128, 128], F32))
    junkm = es.enter_context(nc.sbuf_tensor("junkm", [128, 128], F32))
    partials = es.enter_context(nc.sbuf_tensor("partials", [128, 8], F32))
    gb = es.enter_context(nc.sbuf_tensor("gb", [128, 4], F32))
    gc = es.enter_context(nc.sbuf_tensor("gc", [128, NCLS], F32))
    junk43 = es.enter_context(nc.sbuf_tensor("junk43", [128, NCLS], F32))
    junk4 = es.enter_context(nc.sbuf_tensor("junk4", [128, 4], F32))
    agv = es.enter_context(nc.sbuf_tensor("agv", [8, 8], F32))
    agr = es.enter_context(nc.sbuf_tensor("agr", [8, 8], F32))
    pvec = es.enter_context(nc.sbuf_tensor("pvec", [1, 8], F32))
    res = es.enter_context(nc.sbuf_tensor("res", [1, 1], F32))
    sc = es.enter_context(nc.sbuf_tensor("sc", [128, 40], F32))
    sci = es.enter_context(nc.sbuf_tensor("sci", [128, 4], I32))
    tbox = es.enter_context(nc.sbuf_tensor("tbox", [128, 4], F32))
    gcp = es.enter_context(nc.sbuf_tensor("gcp", [128, NCLS], F32))
    fw0 = es.enter_context(nc.sbuf_tensor("fw0", [128, 400], F32))
    fw1 = es.enter_context(nc.sbuf_tensor("fw1", [128, 400], F32))
    fw2 = es.enter_context(nc.sbuf_tensor("fw2", [128, 400], F32))
    fw3 = es.enter_context(nc.sbuf_tensor("fw3", [128, 400], F32))
    fw4 = es.enter_context(nc.sbuf_tensor("fw4", [128, 400], F32))
    fw5 = es.enter_context(nc.sbuf_tensor("fw5", [128, 400], F32))
    fw6 = es.enter_context(nc.sbuf_tensor("fw6", [128, 400], F32))
    ps0 = es.enter_context(nc.psum_tensor("ps0", [128, 512], F32))
    ps1 = es.enter_context(nc.psum_tensor("ps1", [128, 512], F32))
    ps2 = es.enter_context(nc.psum_tensor("ps2", [128, 512], F32))
    ps3 = es.enter_context(nc.psum_tensor("ps3", [128, 512], F32))
    ps4 = es.enter_context(nc.psum_tensor("ps4", [128, 512], F32))
    ps5 = es.enter_context(nc.psum_tensor("ps5", [128, 512], F32))
    ps6 = es.enter_context(nc.psum_tensor("ps6", [128, 512], F32))
    psp = es.enter_context(nc.psum_tensor("psp", [1, 8], F32))
    with es:
        ps = [ps0, ps1, ps2, ps3, ps4, ps5, ps6]

        # named [128,1] f32 scratch columns
        names = [
            "sumx", "csx", "sumy", "csy", "gxf", "gyf", "bw", "bh", "area",
            "rq", "rm", "rf", "r2", "rr", "nsc", "labf", "vlab", "vbw", "vbh",
            "bbs", "vbs", "valid", "gx2", "gy2", "g2s", "w3a", "pen", "labcf",
            "cellf", "cellgf", "keyf", "cva", "sent", "later1", "kept",
            "later2", "keep2", "l1r", "negrow", "plab",
        ]
        col = {n: sc[:, i : i + 1] for i, n in enumerate(names)}
        idxp1 = cvec[:, 0:1]
        nc.const_aps.aps[(F32, 0.0)] = cvec[:, 3:4]
        nc.const_aps.aps[(F32, 1.0)] = cvec[:, 1:2]
        nc.const_aps.aps[(F32, 1e-6)] = cvec[:, 4:5]
        nc.const_aps.aps[(F32, -LN4)] = cvec[:, 5:6]
        ones = cvec[:, 1:2]
        basef = cvec[:, 2:3]
        gxi = sci[:, 0:1]
        gyi = sci[:, 1:2]
        cellg = sci[:, 2:3]

        with nc.Block() as block:

            @block.sync
            def _(sync):
                sync.dma_start(out=sbb[:, :], in_=bb_d[:, :, :].rearrange("a m c -> (a m) c")).then_inc(dma_in, 16)
                sync.dma_start(out=slab[:, :], in_=lab_d[:, :].rearrange("a m -> (a m)").unsqueeze(1)).then_inc(dma_in, 16)
                sync.dma_start(out=sQ[:, :], in_=qg_d[:, :]).then_inc(dma_in, 16)
                sync.dma_start(out=sU[:, :], in_=ut_d[:, :]).then_inc(dma_in, 16)
                sync.dma_start(out=cvec[:, :], in_=cv_d[:, :]).then_inc(dma_in, 16)
                sync.dma_start(out=chm[:, :], in_=ch_d[:, :]).then_inc(dma_in, 16)
                # hmP[p, 2*fo+img] = hm[img, 128*fo + p]
                sync.dma_start(
                    out=hmP[:, :].rearrange("p (i f) -> p i f", i=2),
                    in_=hm_d[:, :].rearrange("i (p f) -> p i f", p=128),
                ).then_inc(dma_in, 16)
                # broadcast rows 4/5 of tmp via dram bounce
                sync.wait_ge(tr_s, 1)
                sync.dma_start(out=dbc[0:1, :], in_=tmp[4:5, :]).then_inc(dma2, 16)
                sync.dma_start(out=dbc[1:2, :], in_=tmp[5:6, :]).then_inc(dma2, 16)
                sync.wait_ge(dma2, 32)
                sync.dma_start(out=cbc[:, :], in_=dbc[0:1, :].to_broadcast([128, 128])).then_inc(dma2, 16)
                sync.dma_start(out=kbc[:, :], in_=dbc[1:2, :].to_broadcast([128, 128])).then_inc(dma2, 16)
                # partial-sum vector out
                sync.wait_ge(pv_s, 1)
                sync.dma_start(out=cc_in[:, :], in_=pvec[:, :]).then_inc(d4, 16)
                # final result out
                sync.wait_ge(fin_s, 1)
                sync.dma_start(out=out_d[:, :], in_=res[:, :]).then_inc(d6, 16)
                nd6 = 16
                if debug:
                    for nm, t in [("d_partials", partials), ("d_pvec", pvec), ("d_agv", agv),
                                  ("d_sc", sc)]:
                        sync.dma_start(out=dbg[nm][:, :], in_=t[:, :]).then_inc(d6, 16)
                        nd6 += 16
                sync.wait_ge(d6, nd6)

            @block.scalar
            def _(scalar):
                scalar = _DrainAfter(scalar)
                # sqrt(area)/4 = exp(0.5*ln(area) - ln4)
                scalar.wait_ge(va, 1)
                scalar.activation(col["rq"], col["area"], ACT.Ln)
                scalar.activation(col["rq"], col["rq"], ACT.Exp, bias=-LN4, scale=0.5)
                # dw/dh = ln(bw*0.25 + 1e-6)
                scalar.activation(tbox[:, 2:3], col["bw"], ACT.Ln, bias=1e-6, scale=0.25)
                scalar.activation(tbox[:, 3:4], col["bh"], ACT.Ln, bias=1e-6, scale=0.25)
                scalar.sem_inc(av, 1)
                # cls sigmoid via exp
                scalar.wait_ge(g_s, 32)
                scalar.activation(gcp[:, :], gc[:, :], ACT.Exp, scale=-1.0)
                scalar.sem_inc(av, 1)
                # ln(1-p), p^2 for cls neg terms (gcp holds clipped p by now)
                scalar.wait_ge(va, 2)
                scalar.activation(junk43[:, :], gcp[:, :], ACT.Ln, bias=1.0, scale=-1.0)
                scalar.activation(gc[:, :], gcp[:, :], ACT.Square)
                scalar.sem_inc(av, 1)
                # pos-correction transcendentals on plab
                scalar.wait_ge(va, 3)
                scalar.activation(col["sumx"], col["plab"], ACT.Ln)                      # ln p
                scalar.activation(col["sumy"], col["plab"], ACT.Ln, bias=1.0, scale=-1.0)  # ln(1-p)
                scalar.activation(col["gx2"], col["plab"], ACT.Square)                    # p^2
                scalar.activation(col["gy2"], col["plab"], ACT.Square, bias=1.0, scale=-1.0)  # (1-p)^2
                scalar.sem_inc(av, 1)
                # heat: exp of clamped log-heatmap; focal transcendentals
                scalar.wait_ge(va, 4)
                scalar.activation(fw1[:, :], fw0[:, :], ACT.Exp)        # texp from hmLc
                scalar.activation(fw2[:, :], fw6[:, :], ACT.Ln)         # ln p
                scalar.activation(fw3[:, :], fw6[:, :], ACT.Ln, bias=1.0, scale=-1.0)   # ln(1-p)
                scalar.activation(fw4[:, :], fw6[:, :], ACT.Square)     # p^2
                scalar.activation(fw5[:, :], fw6[:, :], ACT.Square, bias=1.0, scale=-1.0)  # (1-p)^2
                scalar.sem_inc(av, 1)

            @block.tensor
            def _(tensor):
                tensor.wait_ge(wrdy, 1)
                tensor.wait_ge(dma_in, 112)
                for g in range(NGRP):
                    b = g % NBANK
                    if g >= NBANK:
                        tensor.wait_ge(dv_s, g - NBANK + 1)
                    for c in range(4):
                        pixel0 = g * 512 + c * 128
                        tensor.matmul(
                            ps[b][:, c * 128 : (c + 1) * 128],
                            sQ[:, pixel0 : pixel0 + 128],
                            tmp[0:4, :],
                            start=True,
                            stop=True,
                        ).then_inc(pe_s, 1)
                # partial-sum reduction over partitions
                tensor.wait_ge(pt_s, 1)
                tensor.matmul(psp[:, :], ones, partials[:, :], start=True, stop=True).then_inc(pp_s, 1)
                tensor.wait_ge(d5, 16)
                tensor.matmul(psp[:, :], ones[0:8], agv[:, :], start=True, stop=True, skip_group_check=True).then_inc(pp_s, 1)

            @block.gpsimd
            def _(gpsimd):
                gpsimd.wait_ge(cell_s, 1)
                gpsimd.indirect_dma_start(
                    out=gb[:, :], out_offset=None,
                    in_=pb_d[:, :],
                    in_offset=bass.IndirectOffsetOnAxis(ap=cellg, axis=0),
                ).then_inc(g_s, 16)
                gpsimd.indirect_dma_start(
                    out=gc[:, :], out_offset=None,
                    in_=pc_d[:, :],
                    in_offset=bass.IndirectOffsetOnAxis(ap=cellg, axis=0),
                ).then_inc(g_s, 16)
                gpsimd.wait_ge(d4, 16)
                gpsimd.collective_compute(
                    "AllGather", ALU.bypass,
                    ins=[cc_in[:, :]], outs=[cc_out[:, :]],
                    replica_groups=[core_ids],
                ).then_inc(cc_s, 1)
                gpsimd.wait_ge(cc_s, 1)
                gpsimd.dma_start(out=agv[:, :], in_=cc_out[:, :]).then_inc(d5, 16)

            @block.vector
            def _(vraw):
                v = _DrainAfter(vraw)
                ts, stt = v.tensor_scalar, v.scalar_tensor_tensor
                x1, y1, x2, y2 = (sbb[:, i : i + 1] for i in range(4))
                v.memset(W5[:, :], 0.0)
                v.wait_ge(dma_in, 112)
                # ---- stage A: per-object quantities ----
                v.tensor_add(col["sumx"], x1, x2)
                ts(col["csx"], col["sumx"], 0.125, 0.5, op0=ALU.mult, op1=ALU.subtract)
                v.tensor_add(col["sumy"], y1, y2)
                ts(col["csy"], col["sumy"], 0.125, 0.5, op0=ALU.mult, op1=ALU.subtract)
                v.tensor_copy(gxi, col["csx"])   # round -> trunc(cx/4)
                v.tensor_copy(col["gxf"], gxi)
                ts(col["gxf"], col["gxf"], 0.0, 159.0, op0=ALU.max, op1=ALU.min)
                v.tensor_copy(gyi, col["csy"])
                v.tensor_copy(col["gyf"], gyi)
                ts(col["gyf"], col["gyf"], 0.0, 159.0, op0=ALU.max, op1=ALU.min)
                v.tensor_sub(col["bw"], x2, x1)
                v.tensor_sub(col["bh"], y2, y1)
                v.tensor_mul(col["area"], col["bw"], col["bh"])
                v.sem_inc(va, 1)                 # scalar: rq, dw, dh
                v.wait_ge(av, 1)
                ts(col["rm"], col["rq"], 2.0, 0.5, op0=ALU.max, op1=ALU.subtract)
                v.tensor_copy(sci[:, 3:4], col["rm"])
                v.tensor_copy(col["rf"], sci[:, 3:4])
                v.tensor_mul(col["r2"], col["rf"], col["rf"])
                v.reciprocal(col["rr"], col["r2"])
                ts(W5[:, 0:1], col["rr"], -2.0, None, op0=ALU.mult)        # nsc
                v.tensor_copy(col["nsc"], W5[:, 0:1])
                # validity
                v.tensor_copy(col["labf"], slab[:, :])
                ts(col["vlab"], col["labf"], 0.0, None, op0=ALU.is_ge)
                ts(col["vbw"], col["bw"], 0.0, None, op0=ALU.is_gt)
                ts(col["vbh"], col["bh"], 0.0, None, op0=ALU.is_gt)
                v.tensor_reduce(out=col["bbs"], in_=sbb[:, :], op=ALU.add, axis=mybir.AxisListType.X)
                ts(col["vbs"], col["bbs"], 0.0, None, op0=ALU.is_gt)
                v.tensor_mul(col["valid"], col["vlab"], col["vbw"])
                v.tensor_mul(col["valid"], col["valid"], col["vbh"])
                v.tensor_mul(col["valid"], col["valid"], col["vbs"])
                # W columns
                v.tensor_mul(col["gx2"], col["gxf"], col["gxf"])
                v.tensor_mul(col["gy2"], col["gyf"], col["gyf"])
                v.tensor_add(col["g2s"], col["gx2"], col["gy2"])
                v.tensor_mul(col["w3a"], col["nsc"], col["g2s"])
                ts(col["pen"], col["valid"], 1.0, 1e30, op0=ALU.subtract, op1=ALU.mult)
                v.tensor_add(W5[:, 3:4], col["w3a"], col["pen"])
                v.tensor_mul(col["gx2"], col["nsc"], col["gxf"])
                ts(W5[:, 1:2], col["gx2"], -2.0, None, op0=ALU.mult)
                v.tensor_mul(col["gy2"], col["nsc"], col["gyf"])
                ts(W5[:, 2:3], col["gy2"], -2.0, None, op0=ALU.mult)
                # cell index (float, exact) then int for gather
                stt(col["cellf"], col["gyf"], 160.0, col["gxf"], op0=ALU.mult, op1=ALU.add)
                v.tensor_add(col["cellgf"], col["cellf"], basef)
                v.tensor_copy(cellg, col["cellgf"])
                v.sem_inc(cell_s, 1)             # gpsimd can gather now
                # key = cell*43 + clipped label
                ts(col["labcf"], col["labf"], 0.0, 42.0, op0=ALU.max, op1=ALU.min)
                stt(col["keyf"], col["cellgf"], 43.0, col["labcf"], op0=ALU.mult, op1=ALU.add)
                # sentinelize invalid rows: cellS = cell*valid - (1-valid)*(1+i)
                v.tensor_mul(col["cva"], col["cellgf"], col["valid"])
                stt(col["sent"], col["valid"], 1.0, idxp1, op0=ALU.subtract, op1=ALU.mult)
                v.tensor_add(W5[:, 4:5], col["cva"], col["sent"])
                v.tensor_mul(col["cva"], col["keyf"], col["valid"])
                v.tensor_add(W5[:, 5:6], col["cva"], col["sent"])
                # transpose W5 -> tmp (rows 0-3 = Wt, 4 = cell row, 5 = key row)
                for c4 in range(4):
                    v.transpose(tmp[0:32, c4 * 32 : (c4 + 1) * 32], W5[c4 * 32 : (c4 + 1) * 32, 0:32])
                v.sem_inc(wrdy, 1)
                v.sem_inc(tr_s, 1)
                # box targets dx, dy
                v.tensor_sub(tbox[:, 0:1], col["csx"], col["gxf"])
                v.tensor_sub(tbox[:, 1:2], col["csy"], col["gyf"])
                # ---- dedup ----
                v.wait_ge(dma2, 64)
                ts(eqt[:, :], cbc[:, :], W5[:, 4:5], None, op0=ALU.is_equal)
                v.tensor_mul(junkm[:, :], eqt[:, :], sU[:, :])
                v.tensor_reduce(out=col["later1"], in_=junkm[:, :], op=ALU.max, axis=mybir.AxisListType.X)
                ts(col["cva"], col["later1"], -1.0, 1.0, op0=ALU.mult, op1=ALU.add)
                v.tensor_mul(partials[:, 2:3], col["valid"], col["cva"])   # kept
                v.tensor_copy(col["kept"], partials[:, 2:3])
                ts(eqt[:, :], kbc[:, :], W5[:, 5:6], None, op0=ALU.is_equal)
                v.tensor_mul(junkm[:, :], eqt[:, :], sU[:, :])
                v.tensor_reduce(out=col["later2"], in_=junkm[:, :], op=ALU.max, axis=mybir.AxisListType.X)
                ts(col["cva"], col["later2"], -1.0, 1.0, op0=ALU.mult, op1=ALU.add)
                v.tensor_mul(partials[:, 5:6], col["valid"], col["cva"])   # keep2
                v.tensor_copy(col["keep2"], partials[:, 5:6])
                # ---- box l1 (gathers needed) ----
                v.wait_ge(g_s, 32)
                v.tensor_sub(junk4[:, :], gb[:, :], tbox[:, :])
                ts(gb[:, :], junk4[:, :], -1.0, None, op0=ALU.mult)
                v.tensor_tensor(junk4[:, :], junk4[:, :], gb[:, :], op=ALU.max)
                v.tensor_reduce(out=col["l1r"], in_=junk4[:, :], op=ALU.add, axis=mybir.AxisListType.X)
                v.tensor_mul(partials[:, 3:4], col["l1r"], col["kept"])
                # ---- cls neg terms ----
                v.wait_ge(av, 2)                   # gcp = exp(-x)
                ts(gcp[:, :], gcp[:, :], 1.0, None, op0=ALU.add)
                v.reciprocal(gcp[:, :], gcp[:, :])
                ts(gcp[:, :], gcp[:, :], EPS, 1.0 - EPS, op0=ALU.max, op1=ALU.min)
                v.sem_inc(va, 1)                  # scalar: ln(1-p), p^2
                v.wait_ge(av, 3)
                stt(junk43[:, :], gc[:, :], -0.75, junk43[:, :], op0=ALU.mult, op1=ALU.mult, accum_out=col["negrow"])
                v.tensor_mul(partials[:, 4:5], col["negrow"], col["kept"])
                # ---- cls pos corrections ----
                ts(eqt[:, 0:NCLS], chm[:, :], col["labcf"], None, op0=ALU.is_equal)
                v.tensor_mul(junk43[:, :], gcp[:, :], eqt[:, 0:NCLS])
                v.tensor_reduce(out=col["plab"], in_=junk43[:, :], op=ALU.add, axis=mybir.AxisListType.X)
                v.sem_inc(va, 1)                  # scalar: ln/sq on plab
                v.wait_ge(av, 4)
                # pos_t = -0.25*(1-p)^2*ln p ; neg_t = -0.75*p^2*ln(1-p)
                stt(col["cva"], col["gy2"], -0.25, col["sumx"], op0=ALU.mult, op1=ALU.mult)
                stt(col["sent"], col["gx2"], -0.75, col["sumy"], op0=ALU.mult, op1=ALU.mult)
                v.tensor_sub(col["cva"], col["cva"], col["sent"])
                v.tensor_mul(partials[:, 6:7], col["cva"], col["keep2"])
                v.memset(partials[:, 7:8], 0.0)
                # ---- heatmap max-reduce pipeline ----
                last_inc = 0
                for g in range(NGRP):
                    b = g % NBANK
                    v.wait_ge(pe_s, 4 * (g + 1))
                    vraw.tensor_reduce(
                        out=hmL[:, :].rearrange("p (i f) -> p f i", i=2)[:, 4 * g : 4 * g + 4, :],
                        in_=ps[b][:, :].rearrange("p (a b m) -> p a b m", a=4, b=2),
                        op=ALU.max,
                        axis=mybir.AxisListType.X,
                    )
                    vraw.drain().then_inc(dv_s, 1)
                # ---- heat focal ----
                ts(fw0[:, :], hmL[:, :], -80.0, None, op0=ALU.max)          # hmLc
                ts(fw6[:, :], hmP[:, :], EPS, 1.0 - EPS, op0=ALU.max, op1=ALU.min)  # p
                v.sem_inc(va, 1)                  # scalar: texp, ln p, ln(1-p), p^2, (1-p)^2
                ts(hmP[:, :], fw0[:, :], THR, None, op0=ALU.is_ge)          # keep mask (reuse hmP)
                v.wait_ge(av, 5)
                v.tensor_mul(fw1[:, :], fw1[:, :], hmP[:, :])              # t
                stt(fw2[:, :], fw5[:, :], -0.25, fw2[:, :], op0=ALU.mult, op1=ALU.mult)  # A
                stt(fw3[:, :], fw4[:, :], 0.75, fw3[:, :], op0=ALU.mult, op1=ALU.mult)   # B'
                v.tensor_mul(fw4[:, :], fw2[:, :], fw1[:, :])              # X = A*t
                stt(fw5[:, :], fw1[:, :], 1.0, fw3[:, :], op0=ALU.subtract, op1=ALU.mult)  # Y = (t-1)*B'
                ts(fw0[:, :], fw1[:, :], 0.5, None, op0=ALU.is_gt)  # pos
                v.tensor_reduce(out=partials[:, 0:1], in_=fw0[:, :], op=ALU.add, axis=mybir.AxisListType.X)
                v.tensor_sub(fw2[:, :], fw4[:, :], fw5[:, :])          # X - Y
                v.tensor_mul(fw2[:, :], fw2[:, :], fw0[:, :])          # (X-Y)*pos
                v.tensor_add(fw6[:, :], fw2[:, :], fw5[:, :])
                v.tensor_reduce(out=partials[:, 1:2], in_=fw6[:, :], op=ALU.add, axis=mybir.AxisListType.X)
                v.sem_inc(pt_s, 1)
                # ---- partial vec out, collective, final ----
                v.wait_ge(pp_s, 1)
                v.tensor_copy(pvec[:, :], psp[:, :])
                v.sem_inc(pv_s, 1)
                v.wait_ge(pp_s, 2)
                v.tensor_copy(pvec[:, :], psp[:, :])
                gcol = [pvec[0:1, i : i + 1] for i in range(8)]
                r0, r1, r2 = sc[0:1, 0:1], sc[0:1, 1:2], sc[0:1, 2:3]
                r3, r4, r5 = sc[0:1, 3:4], sc[0:1, 4:5], sc[0:1, 5:6]
                ts(r0, gcol[0], 1.0, None, op0=ALU.max)
                v.reciprocal(r0, r0)
                v.tensor_mul(r0, gcol[1], r0)                    # heat_loss
                ts(r1, gcol[2], 1.0, None, op0=ALU.max)
                v.reciprocal(r1, r1)
                ts(r2, gcol[2], 1.0, None, op0=ALU.is_gt)         # ind
                v.tensor_mul(r1, gcol[3], r1)
                v.tensor_mul(r1, r1, r2)                         # box_loss
                ts(r3, gcol[5], 1.0, None, op0=ALU.max)
                v.reciprocal(r3, r3)
                v.tensor_add(r4, gcol[4], gcol[6])
                v.tensor_mul(r3, r4, r3)
                v.tensor_mul(r3, r3, r2)                         # cls_loss
                v.tensor_add(r5, r0, r1)
                v.tensor_add(res[:, :], r5, r3)
                v.sem_inc(fin_s, 1)

    return nc


_CACHE = {}


def _consts():
    j = np.arange(PIX)
    pix = (j % 128) * 200 + 4 * (j // 512) + (j % 512) // 128
    x = (pix % W).astype(np.float32)
    y = (pix // W).astype(np.float32)
    qgrid = np.stack([x * x + y * y, x, y, np.ones_like(x)]).astype(np.float32)
    utri = np.triu(np.ones((128, 128), dtype=np.float32), k=1)
    cvec = np.zeros((128, 8), dtype=np.float32)
    cvec[:, 0] = np.arange(128) + 1.0
    cvec[:, 1] = 1.0
    cvec[64:, 2] = PIX
    cvec[:, 4] = 1e-6
    cvec[:, 5] = -LN4
    chm = np.broadcast_to(np.arange(NCLS, dtype=np.float32), (128, NCLS)).copy()
    return qgrid, utri, cvec, chm


def kernel(pred_heatmap, pred_boxes, pred_classes, bboxes, labels):
    if "nc" not in _CACHE:
        _CACHE["nc"] = _build()
    nc = _CACHE["nc"]

    qgrid, utri, cvec, chm = _consts()
    pbt = np.ascontiguousarray(pred_boxes.transpose(0, 2, 3, 1).reshape(B, PIX, 4))
    pct = np.ascontiguousarray(pred_classes.transpose(0, 2, 3, 1).reshape(B, PIX, NCLS))
    hmf = np.ascontiguousarray(pred_heatmap.reshape(B, PIX)).astype(np.float32)
    lab32 = labels.astype(np.int32)

    in_maps = []
    for c in range(NC):
        s = slice(c * BPC, (c + 1) * BPC)
        in_maps.append({
            "hm": hmf[s],
            "pbt": pbt[s].reshape(BPC * PIX, 4),
            "pct": pct[s].reshape(BPC * PIX, NCLS),
            "bb": np.ascontiguousarray(bboxes[s]).astype(np.float32),
            "lab": np.ascontiguousarray(lab32[s]),
            "qgrid": qgrid, "utri": utri, "cvec": cvec, "chm": chm,
        })

    r = run_bass_kernel_spmd(nc, in_maps, list(range(NC)))
    return np.float32(np.asarray(r.results[0]["out"]).reshape(-1)[0])


if __name__ == "__main__":
    import reference
    inputs = reference.setup_inputs()
    inputs = {k: np.asarray(v) for k, v in inputs.items()}
    out = kernel(**inputs)
    exp = np.asarray(reference.reference(**{k: v for k, v in inputs.items()}))
    rel = abs(out - exp) / max(abs(exp), 1e-9)
    print("expected:", exp, "actual:", out, "rel:", rel)



# revision 2
# speedup vs baseline: 1.0401x; 1.0401x over previous
"""AnchorFreeLoss on 8 TRN2 NeuronCores — v4.

On top of v3 (fp32r 512-wide matmuls, packed DMA, paired PSUM reduces,
host-side final combine):
- pk split: stage-A columns [128,56] land ~4us before the bulk (utri/hmP).
- DVE program reordered: pair reduces begin as soon as the PE fills the
  first pair; dedup/l1/cls blocks slot between early reduces instead of
  blocking the reduce stream.
- Focal restructured: p-derived planes A=-0.25(1-p)^2 ln p and
  B'=0.75 p^2 ln(1-p) are computed from the input heatmap before the
  reduce stream; pos is computed in log domain (no exp dependency); the
  pos-weighted sum uses stt accum_out. Tail after the last reduce is
  ~6 DVE ops + one scalar exp.
- Optional fp16 staging (USE_F16): gpsimd cast-DMAs each PSUM pair to
  fp16 SBUF; DVE reduces from SBUF at the 2x/4x DVE rate.
"""

import sys
from contextlib import ExitStack

import numpy as np

if "/opt/trn_rl_repo" not in sys.path:
    sys.path.insert(0, "/opt/trn_rl_repo")

from concourse import bass, mybir
from concourse.bass_utils import run_bass_kernel_spmd

F32 = mybir.dt.float32
F32R = mybir.dt.float32r
F16 = mybir.dt.float16
I32 = mybir.dt.int32
ALU = mybir.AluOpType
ACT = mybir.ActivationFunctionType

B, M, H, W = 16, 64, 160, 160
NC = 8
BPC = B // NC
PIX = H * W
NCLS = 43
EPS = 1e-7
LN4 = 1.3862943611198906
LNH = -0.6931471805599453   # ln(0.5)
THR = -8.0
NBANK = 50
NPAIR = 25
PDEPTH = 3
CTR = 80.0

USE_F16 = False

# pkA columns (stage-A critical)
PK_BB = 0
PK_LAB = 4
PK_CV = 5
PK_CHM = 13
PKA_N = 56
# pkB columns (bulk)
PKB_UT = 0
PKB_HM = 128
PKB_N = 528


def _build(debug=False):
    nc = bass.Bass()

    pka_d = nc.declare_dram_parameter("pka", [128, PKA_N], F32, isOutput=False)
    pkb_d = nc.declare_dram_parameter("pkb", [128, PKB_N], F32, isOutput=False)
    q2_d = nc.declare_dram_parameter("q2", [16, 6912], F32R, isOutput=False)
    pb_d = nc.declare_dram_parameter("pbt", [BPC * PIX, 4], F32, isOutput=False)
    pc_d = nc.declare_dram_parameter("pct", [BPC * PIX, NCLS], F32, isOutput=False)
    out_d = nc.declare_dram_parameter("out", [1, 8], F32, isOutput=True)
    dbg = {}
    if debug:
        for nm, shp in [("d_partials", [128, 8]), ("d_sc", [128, 48]),
                        ("d_hmL", [128, 400]), ("d_W5", [128, 32])]:
            dbg[nm] = nc.declare_dram_parameter(nm, shp, F32, isOutput=True)

    dbc = nc.dram_tensor("dbc", [2, 128], F32)

    es = ExitStack()
    dma_in = es.enter_context(nc.semaphore("dma_in"))
    dma_pk = es.enter_context(nc.semaphore("dma_pk"))
    dma_pb = es.enter_context(nc.semaphore("dma_pb"))
    dma2 = es.enter_context(nc.semaphore("dma2"))
    d6 = es.enter_context(nc.semaphore("d6"))
    va = es.enter_context(nc.semaphore("va"))
    vf = es.enter_context(nc.semaphore("vf"))
    av = es.enter_context(nc.semaphore("av"))
    wsem = es.enter_context(nc.semaphore("wsem"))
    tr_s = es.enter_context(nc.semaphore("tr_s"))
    tr2_s = es.enter_context(nc.semaphore("tr2_s"))
    pe_s = es.enter_context(nc.semaphore("pe_s"))
    dv_s = es.enter_context(nc.semaphore("dv_s"))
    st_s = es.enter_context(nc.semaphore("st_s"))
    cell_s = es.enter_context(nc.semaphore("cell_s"))
    g_s = es.enter_context(nc.semaphore("g_s"))
    pt_s = es.enter_context(nc.semaphore("pt_s"))
    pp_s = es.enter_context(nc.semaphore("pp_s"))
    pv_s = es.enter_context(nc.semaphore("pv_s"))
    pka = es.enter_context(nc.sbuf_tensor("pka_s", [128, PKA_N], F32))
    pkb = es.enter_context(nc.sbuf_tensor("pkb_s", [128, PKB_N], F32))
    sQ2 = es.enter_context(nc.sbuf_tensor("sQ2", [16, 6400], F32R))
    blkW = es.enter_context(nc.sbuf_tensor("blkW", [16, 512], F32R))
    W5 = es.enter_context(nc.sbuf_tensor("W5", [128, 32], F32))
    tmpT = es.enter_context(nc.sbuf_tensor("tmpT", [32, 128], F32))
    tmpT2 = es.enter_context(nc.sbuf_tensor("tmpT2", [32, 128], F32))
    sc = es.enter_context(nc.sbuf_tensor("sc", [128, 48], F32))
    sci = es.enter_context(nc.sbuf_tensor("sci", [128, 4], I32))
    if USE_F16:
        hmL = es.enter_context(nc.sbuf_tensor("hmL", [128, 400], F16))
        stg = es.enter_context(nc.sbuf_tensor("stg", [128, PDEPTH * 1024], F16))
    else:
        hmL = es.enter_context(nc.sbuf_tensor("hmL", [128, 400], F32))
        stg = None
    fw0 = es.enter_context(nc.sbuf_tensor("fw0", [128, 400], F32))
    fw1 = es.enter_context(nc.sbuf_tensor("fw1", [128, 400], F32))
    fw2 = es.enter_context(nc.sbuf_tensor("fw2", [128, 400], F32))
    fw3 = es.enter_context(nc.sbuf_tensor("fw3", [128, 400], F32))
    fw4 = es.enter_context(nc.sbuf_tensor("fw4", [128, 400], F32))
    fw5 = es.enter_context(nc.sbuf_tensor("fw5", [128, 400], F32))
    fw6 = es.enter_context(nc.sbuf_tensor("fw6", [128, 400], F32))
    cbc = es.enter_context(nc.sbuf_tensor("cbc", [128, 128], F32))
    kbc = es.enter_context(nc.sbuf_tensor("kbc", [128, 128], F32))
    eqt = es.enter_context(nc.sbuf_tensor("eqt", [128, 128], F32))
    junkm = es.enter_context(nc.sbuf_tensor("junkm", [128, 128], F32))
    partials = es.enter_context(nc.sbuf_tensor("partials", [128, 8], F32))
    gb = es.enter_context(nc.sbuf_tensor("gb", [128, 4], F32))
    gc = es.enter_context(nc.sbuf_tensor("gc", [128, NCLS], F32))
    gcp = es.enter_context(nc.sbuf_tensor("gcp", [128, NCLS], F32))
    junk43 = es.enter_context(nc.sbuf_tensor("junk43", [128, NCLS], F32))
    junk4 = es.enter_context(nc.sbuf_tensor("junk4", [128, 4], F32))
    tbox = es.enter_context(nc.sbuf_tensor("tbox", [128, 4], F32))
    pvec = es.enter_context(nc.sbuf_tensor("pvec", [1, 8], F32))
    pp0 = es.enter_context(nc.psum_tensor("pp0", [128, 1024], F32))
    pp1 = es.enter_context(nc.psum_tensor("pp1", [128, 1024], F32))
    pp2 = es.enter_context(nc.psum_tensor("pp2", [128, 1024], F32))
    psp = es.enter_context(nc.psum_tensor("psp", [1, 8], F32))
    with es:
        pp = [pp0, pp1, pp2]

        names = [
            "sumx", "sumy", "csx", "csy", "gxf", "gyf", "bw", "bh", "bbs",
            "vbw", "vbh", "vbs", "vlab", "valid", "rq", "rm", "rf", "r2",
            "rr", "gxc", "gyc", "gx2", "gy2", "g2s", "w3a", "pen",
            "t2a", "t2b", "cellf", "cellgf", "labcf", "keyf",
            "cva", "sent", "kept", "keep2", "later1", "later2",
            "l1r", "negrow", "plab", "lnp", "ln1mp", "psq", "mpsq", "sumy2",
        ]
        col = {n: sc[:, i: i + 1] for i, n in enumerate(names)}

        idxp1 = pka[:, PK_CV + 0: PK_CV + 1]
        ones = pka[:, PK_CV + 1: PK_CV + 2]
        basef = pka[:, PK_CV + 2: PK_CV + 3]
        nc.const_aps.aps[(F32, 0.0)] = pka[:, PK_CV + 3: PK_CV + 4]
        nc.const_aps.aps[(F32, 1.0)] = ones
        nc.const_aps.aps[(F32, 1e-6)] = pka[:, PK_CV + 4: PK_CV + 5]
        nc.const_aps.aps[(F32, -LN4)] = pka[:, PK_CV + 5: PK_CV + 6]
        labf = pka[:, PK_LAB: PK_LAB + 1]
        chm = pka[:, PK_CHM: PK_CHM + NCLS]
        utri = pkb[:, PKB_UT: PKB_UT + 128]
        pkhm = pkb[:, PKB_HM: PKB_HM + 400]
        cellg = sci[:, 2:3]

        with nc.Block() as block:

            @block.sync
            def _(sync):
                sync.dma_start(out=pka[:, :], in_=pka_d[:, :]).then_inc(dma_pk, 16)
                sync.dma_start(out=sQ2[:, :], in_=q2_d[:, 0:6400]).then_inc(dma_in, 16)
                sync.dma_start(out=blkW[:, :], in_=q2_d[:, 6400:6912]).then_inc(dma_in, 16)
                sync.dma_start(out=pkb[:, :], in_=pkb_d[:, :]).then_inc(dma_pb, 16)
                sync.wait_ge(tr2_s, 1)
                sync.dma_start(out=dbc[0:1, :], in_=tmpT2[4:5, :]).then_inc(dma2, 16)
                sync.dma_start(out=dbc[1:2, :], in_=tmpT2[5:6, :]).then_inc(dma2, 16)
                sync.wait_ge(dma2, 32)
                sync.dma_start(out=cbc[:, :], in_=dbc[0:1, :].to_broadcast([128, 128])).then_inc(dma2, 16)
                sync.dma_start(out=kbc[:, :], in_=dbc[1:2, :].to_broadcast([128, 128])).then_inc(dma2, 16)
                # partial-sum vector out (host combines across cores)
                sync.wait_ge(pv_s, 1)
                sync.dma_start(out=out_d[:, :], in_=pvec[:, :]).then_inc(d6, 16)
                nd6 = 16
                if debug:
                    for nm, t in [("d_partials", partials), ("d_sc", sc),
                                  ("d_hmL", hmL), ("d_W5", W5)]:
                        sync.dma_start(out=dbg[nm][:, :], in_=t[:, :]).then_inc(d6, 16)
                        nd6 += 16
                sync.wait_ge(d6, nd6)

            @block.scalar
            def _(scalar):
                scalar.wait_ge(va, 1)
                scalar.activation(col["rq"], col["bbs"], ACT.Ln)
                scalar.drain()
                scalar.activation(col["rq"], col["rq"], ACT.Exp, bias=-LN4, scale=0.5)
                scalar.activation(tbox[:, 2:4], sc[:, 6:8], ACT.Ln, bias=1e-6, scale=0.25)
                scalar.drain()
                scalar.sem_inc(av, 1)
                # early focal transcendentals from p (input-only)
                scalar.wait_ge(vf, 1)
                scalar.activation(fw2[:, :], fw6[:, :], ACT.Ln)
                scalar.activation(fw3[:, :], fw6[:, :], ACT.Ln, bias=1.0, scale=-1.0)
                scalar.activation(fw4[:, :], fw6[:, :], ACT.Square)
                scalar.activation(fw5[:, :], fw6[:, :], ACT.Square, bias=1.0, scale=-1.0)
                scalar.drain()
                scalar.sem_inc(av, 1)
                # cls sigmoid via exp
                scalar.wait_ge(g_s, 32)
                scalar.activation(gcp[:, :], gc[:, :], ACT.Exp, scale=-1.0)
                scalar.drain()
                scalar.sem_inc(av, 1)
                scalar.wait_ge(va, 2)
                scalar.activation(junk43[:, :], gcp[:, :], ACT.Ln, bias=1.0, scale=-1.0)
                scalar.activation(gc[:, :], gcp[:, :], ACT.Square)
                scalar.drain()
                scalar.sem_inc(av, 1)
                scalar.wait_ge(va, 3)
                scalar.activation(col["lnp"], col["plab"], ACT.Ln)
                scalar.activation(col["ln1mp"], col["plab"], ACT.Ln, bias=1.0, scale=-1.0)
                scalar.activation(col["psq"], col["plab"], ACT.Square)
                scalar.activation(col["mpsq"], col["plab"], ACT.Square, bias=1.0, scale=-1.0)
                scalar.drain()
                scalar.sem_inc(av, 1)
                # t = exp(clamped log heatmap)
                scalar.wait_ge(va, 4)
                scalar.activation(fw1[:, :], fw0[:, :], ACT.Exp)
                scalar.drain()
                scalar.sem_inc(av, 1)

            @block.tensor
            def _(tensor):
                tensor.wait_ge(wsem, 64)
                tensor.wait_ge(dma_in, 32)
                for g in range(NBANK):
                    gp = g // 2
                    pt = pp[gp % PDEPTH]
                    half = (g % 2) * 512
                    if gp >= PDEPTH:
                        tensor.wait_ge(dv_s, gp - PDEPTH + 1)
                    tensor.matmul(
                        pt[:, half: half + 512],
                        sQ2[:, g * 128: (g + 1) * 128],
                        blkW[:, :],
                        start=True,
                        stop=True,
                    ).then_inc(pe_s, 1)
                tensor.wait_ge(pt_s, 1)
                tensor.matmul(psp[:, :], ones, partials[:, :], start=True, stop=True).then_inc(pp_s, 1)

            @block.gpsimd
            def _(gpsimd):
                gpsimd.wait_ge(dma_in, 32)
                gpsimd.wait_ge(tr_s, 1)
                for c in range(4):
                    gpsimd.dma_start(
                        out=blkW[4 * c: 4 * c + 4, c * 128: (c + 1) * 128],
                        in_=tmpT[0:4, :],
                    ).then_inc(wsem, 16)
                gpsimd.wait_ge(cell_s, 1)
                gpsimd.indirect_dma_start(
                    out=gb[:, :], out_offset=None,
                    in_=pb_d[:, :],
                    in_offset=bass.IndirectOffsetOnAxis(ap=cellg, axis=0),
                ).then_inc(g_s, 16)
                gpsimd.indirect_dma_start(
                    out=gc[:, :], out_offset=None,
                    in_=pc_d[:, :],
                    in_offset=bass.IndirectOffsetOnAxis(ap=cellg, axis=0),
                ).then_inc(g_s, 16)
                if USE_F16:
                    # fp16 cast staging of each PSUM pair (frees the bank for PE)
                    for gp in range(NPAIR):
                        gpsimd.wait_ge(pe_s, 2 * (gp + 1))
                        gpsimd.dma_start(
                            out=stg[:, (gp % PDEPTH) * 1024: (gp % PDEPTH) * 1024 + 1024],
                            in_=pp[gp % PDEPTH][:, :],
                        ).then_inc(st_s, 16)

            @block.vector
            def _(v):
                ts, stt = v.tensor_scalar, v.scalar_tensor_tensor

                def D():
                    v.drain()

                v.wait_ge(dma_pk, 16)
                # ---- stage A ----
                v.tensor_add(sc[:, 0:2], pka[:, 0:2], pka[:, 2:4])
                D()
                ts(sc[:, 2:4], sc[:, 0:2], 0.125, 0.5, op0=ALU.mult, op1=ALU.subtract)
                D()
                v.tensor_copy(sci[:, 0:2], sc[:, 2:4])
                D()
                v.tensor_copy(sc[:, 4:6], sci[:, 0:2])
                D()
                ts(sc[:, 4:6], sc[:, 4:6], 0.0, 159.0, op0=ALU.max, op1=ALU.min)
                D()
                v.tensor_sub(sc[:, 6:8], pka[:, 2:4], pka[:, 0:2])
                D()
                v.tensor_mul(col["bbs"], col["bw"], col["bh"])
                D()
                v.sem_inc(va, 1)
                v.tensor_reduce(out=col["rm"], in_=pka[:, 0:4], op=ALU.add, axis=mybir.AxisListType.X)
                D()
                ts(sc[:, 9:11], sc[:, 6:8], 0.0, None, op0=ALU.is_gt)
                D()
                ts(col["vbs"], col["rm"], 0.0, None, op0=ALU.is_gt)
                D()
                ts(col["vlab"], labf, 0.0, None, op0=ALU.is_ge)
                D()
                v.tensor_reduce(out=col["valid"], in_=sc[:, 9:13], op=ALU.mult, axis=mybir.AxisListType.X)
                D()
                ts(sc[:, 19:21], sc[:, 4:6], CTR, None, op0=ALU.subtract)
                D()
                v.tensor_mul(sc[:, 21:23], sc[:, 19:21], sc[:, 19:21])
                D()
                v.tensor_add(col["g2s"], col["gx2"], col["gy2"])
                D()
                ts(col["pen"], col["valid"], 1.0, 1e30, op0=ALU.subtract, op1=ALU.mult)
                D()
                v.wait_ge(av, 1)
                ts(col["rm"], col["rq"], 2.0, 0.5, op0=ALU.max, op1=ALU.subtract)
                D()
                v.tensor_copy(sci[:, 3:4], col["rm"])
                D()
                v.tensor_copy(col["rf"], sci[:, 3:4])
                D()
                v.tensor_mul(col["r2"], col["rf"], col["rf"])
                D()
                v.reciprocal(col["rr"], col["r2"])
                D()
                ts(W5[:, 0:1], col["rr"], -2.0, None, op0=ALU.mult)
                D()
                v.tensor_mul(col["w3a"], W5[:, 0:1], col["g2s"])
                D()
                v.tensor_add(W5[:, 3:4], col["w3a"], col["pen"])
                D()
                ts(sc[:, 26:28], sc[:, 19:21], W5[:, 0:1], None, op0=ALU.mult)
                D()
                ts(W5[:, 1:3], sc[:, 26:28], -2.0, None, op0=ALU.mult)
                D()
                for c4 in range(4):
                    v.transpose(tmpT[0:32, c4 * 32: (c4 + 1) * 32], W5[c4 * 32: (c4 + 1) * 32, 0:32])
                D()
                v.sem_inc(tr_s, 1)
                # cell/key
                stt(col["cellf"], col["gyf"], 160.0, col["gxf"], op0=ALU.mult, op1=ALU.add)
                D()
                v.tensor_add(col["cellgf"], col["cellf"], basef)
                D()
                v.tensor_copy(cellg, col["cellgf"])
                D()
                v.sem_inc(cell_s, 1)
                ts(col["labcf"], labf, 0.0, 42.0, op0=ALU.max, op1=ALU.min)
                D()
                stt(col["keyf"], col["cellgf"], 43.0, col["labcf"], op0=ALU.mult, op1=ALU.add)
                D()
                v.tensor_mul(col["cva"], col["cellgf"], col["valid"])
                D()
                stt(col["sent"], col["valid"], 1.0, idxp1, op0=ALU.subtract, op1=ALU.mult)
                D()
                v.tensor_add(W5[:, 4:5], col["cva"], col["sent"])
                D()
                v.tensor_mul(col["cva"], col["keyf"], col["valid"])
                D()
                v.tensor_add(W5[:, 5:6], col["cva"], col["sent"])
                D()
                for c4 in range(4):
                    v.transpose(tmpT2[0:32, c4 * 32: (c4 + 1) * 32], W5[c4 * 32: (c4 + 1) * 32, 0:32])
                D()
                v.sem_inc(tr2_s, 1)
                v.tensor_sub(tbox[:, 0:2], sc[:, 2:4], sc[:, 4:6])
                D()
                # ---- focal prelude from input heatmap ----
                v.wait_ge(dma_pb, 16)
                ts(fw6[:, :], pkhm, EPS, 1.0 - EPS, op0=ALU.max, op1=ALU.min)  # p
                D()
                v.sem_inc(vf, 1)      # scalar: fw2..fw5
                # ---- dedup ----
                v.wait_ge(dma2, 64)
                ts(eqt[:, :], cbc[:, :], W5[:, 4:5], None, op0=ALU.is_equal)
                D()
                v.tensor_mul(junkm[:, :], eqt[:, :], utri)
                D()
                v.tensor_reduce(out=col["later1"], in_=junkm[:, :], op=ALU.max, axis=mybir.AxisListType.X)
                D()
                ts(col["cva"], col["later1"], -1.0, 1.0, op0=ALU.mult, op1=ALU.add)
                D()
                v.tensor_mul(partials[:, 2:3], col["valid"], col["cva"])
                D()
                v.tensor_copy(col["kept"], partials[:, 2:3])
                D()
                ts(eqt[:, :], kbc[:, :], W5[:, 5:6], None, op0=ALU.is_equal)
                D()
                v.tensor_mul(junkm[:, :], eqt[:, :], utri)
                D()
                v.tensor_reduce(out=col["later2"], in_=junkm[:, :], op=ALU.max, axis=mybir.AxisListType.X)
                D()
                ts(col["cva"], col["later2"], -1.0, 1.0, op0=ALU.mult, op1=ALU.add)
                D()
                v.tensor_mul(partials[:, 5:6], col["valid"], col["cva"])
                D()
                v.tensor_copy(col["keep2"], partials[:, 5:6])
                D()

                def reduce_pair(gp):
                    if USE_F16:
                        v.wait_ge(st_s, 16 * (gp + 1))
                        src = stg[:, (gp % PDEPTH) * 1024: (gp % PDEPTH) * 1024 + 1024]
                    else:
                        v.wait_ge(pe_s, 2 * (gp + 1))
                        src = pp[gp % PDEPTH][:, :]
                    v.tensor_reduce(
                        out=hmL[:, :].rearrange("p (i f) -> p f i", i=2)[:, 8 * gp: 8 * gp + 8, :],
                        in_=src.rearrange("p (a b m) -> p a b m", a=8, b=2, m=64),
                        op=ALU.max,
                        axis=mybir.AxisListType.X,
                    )
                    v.drain().then_inc(dv_s, 1)

                nxt = 0
                for _ in range(3):
                    reduce_pair(nxt); nxt += 1
                # ---- box l1 ----
                v.wait_ge(g_s, 32)
                v.tensor_sub(junk4[:, :], gb[:, :], tbox[:, :])
                D()
                ts(gb[:, :], junk4[:, :], -1.0, None, op0=ALU.mult)
                D()
                v.tensor_tensor(junk4[:, :], junk4[:, :], gb[:, :], op=ALU.max)
                D()
                v.tensor_reduce(out=col["l1r"], in_=junk4[:, :], op=ALU.add, axis=mybir.AxisListType.X)
                D()
                v.tensor_mul(partials[:, 3:4], col["l1r"], col["kept"])
                D()
                for _ in range(2):
                    reduce_pair(nxt); nxt += 1
                # ---- cls neg ----
                v.wait_ge(av, 3)
                ts(gcp[:, :], gcp[:, :], 1.0, None, op0=ALU.add)
                D()
                v.reciprocal(gcp[:, :], gcp[:, :])
                D()
                ts(gcp[:, :], gcp[:, :], EPS, 1.0 - EPS, op0=ALU.max, op1=ALU.min)
                D()
                v.sem_inc(va, 1)
                for _ in range(2):
                    reduce_pair(nxt); nxt += 1
                v.wait_ge(av, 4)
                stt(junk43[:, :], gc[:, :], -0.75, junk43[:, :], op0=ALU.mult, op1=ALU.mult, accum_out=col["negrow"])
                D()
                v.tensor_mul(partials[:, 4:5], col["negrow"], col["kept"])
                D()
                # ---- cls pos ----
                ts(eqt[:, 0:NCLS], chm, col["labcf"], None, op0=ALU.is_equal)
                D()
                v.tensor_mul(junk43[:, :], gcp[:, :], eqt[:, 0:NCLS])
                D()
                v.tensor_reduce(out=col["plab"], in_=junk43[:, :], op=ALU.add, axis=mybir.AxisListType.X)
                D()
                v.sem_inc(va, 1)
                for _ in range(2):
                    reduce_pair(nxt); nxt += 1
                v.wait_ge(av, 5)
                stt(col["cva"], col["mpsq"], -0.25, col["lnp"], op0=ALU.mult, op1=ALU.mult)
                D()
                stt(col["sent"], col["psq"], -0.75, col["ln1mp"], op0=ALU.mult, op1=ALU.mult)
                D()
                v.tensor_sub(col["cva"], col["cva"], col["sent"])
                D()
                v.tensor_mul(partials[:, 6:7], col["cva"], col["keep2"])
                D()
                v.memset(partials[:, 7:8], 0.0)
                D()
                # focal planes A (fw2) and B' (fw3) — input-only, hide before reduces
                v.wait_ge(av, 2)
                stt(fw2[:, :], fw5[:, :], -0.25, fw2[:, :], op0=ALU.mult, op1=ALU.mult)  # A
                D()
                stt(fw3[:, :], fw4[:, :], 0.75, fw3[:, :], op0=ALU.mult, op1=ALU.mult)   # B'
                D()
                # ---- remaining reduce pairs ----
                while nxt < NPAIR:
                    reduce_pair(nxt); nxt += 1
                # ---- heat focal tail ----
                ts(fw0[:, :], hmL[:, :], -80.0, None, op0=ALU.max)
                D()
                v.sem_inc(va, 1)                    # scalar: fw1 = exp(fw0)
                ts(fw4[:, :], fw0[:, :], THR, None, op0=ALU.is_ge)       # keep mask
                D()
                # pos in log domain: (fw0 > ln 0.5) * mask
                stt(fw5[:, :], fw0[:, :], LNH, fw4[:, :], op0=ALU.is_gt, op1=ALU.mult)
                D()
                v.tensor_reduce(out=partials[:, 0:1], in_=fw5[:, :], op=ALU.add, axis=mybir.AxisListType.X)
                D()
                v.wait_ge(av, 6)
                v.tensor_mul(fw1[:, :], fw1[:, :], fw4[:, :])            # t
                D()
                v.tensor_mul(fw4[:, :], fw2[:, :], fw1[:, :])            # X = A*t
                D()
                stt(fw6[:, :], fw1[:, :], 1.0, fw3[:, :], op0=ALU.subtract, op1=ALU.mult,
                    accum_out=col["sumy2"])                              # Y, sum(Y)
                D()
                v.tensor_sub(fw4[:, :], fw4[:, :], fw6[:, :])            # X - Y
                D()
                stt(fw4[:, :], fw4[:, :], 1.0, fw5[:, :], op0=ALU.mult, op1=ALU.mult,
                    accum_out=col["cva"])                                # (X-Y)*pos, sum
                D()
                v.tensor_add(partials[:, 1:2], col["cva"], col["sumy2"])
                D()
                v.sem_inc(pt_s, 1)
                v.wait_ge(pp_s, 1)
                v.tensor_copy(pvec[:, :], psp[:, :])
                D()
                v.sem_inc(pv_s, 1)

    return nc


_CACHE = {}


def _consts():
    p = np.arange(128)
    g = np.arange(NBANK)
    qg2 = np.zeros((16, 6912), np.float32)
    for c in range(4):
        pix = p[None, :] * 200 + 4 * g[:, None] + c
        xx = (pix % W).astype(np.float32) - CTR
        yy = (pix // W).astype(np.float32) - CTR
        q4 = np.stack([xx * xx + yy * yy, xx, yy, np.ones_like(xx)])
        qg2[4 * c: 4 * c + 4, 0:6400] = q4.reshape(4, 6400)
    utri = np.triu(np.ones((128, 128), dtype=np.float32), k=1)
    cvec = np.zeros((128, 8), dtype=np.float32)
    cvec[:, 0] = np.arange(128) + 1.0
    cvec[:, 1] = 1.0
    cvec[64:, 2] = PIX
    cvec[:, 4] = 1e-6
    cvec[:, 5] = -LN4
    chm = np.broadcast_to(np.arange(NCLS, dtype=np.float32), (128, NCLS))
    return qg2, utri, cvec, chm


def _pack(bb, lab32, hmf, utri, cvec, chm):
    pka = np.zeros((128, PKA_N), dtype=np.float32)
    pka[:, PK_BB: PK_BB + 4] = bb.reshape(128, 4)
    pka[:, PK_LAB] = lab32.reshape(128).astype(np.float32)
    pka[:, PK_CV: PK_CV + 8] = cvec
    pka[:, PK_CHM: PK_CHM + NCLS] = chm
    pkb = np.zeros((128, PKB_N), dtype=np.float32)
    pkb[:, PKB_UT: PKB_UT + 128] = utri
    pkb[:, PKB_HM: PKB_HM + 400] = hmf.reshape(BPC, 128, 200).transpose(1, 0, 2).reshape(128, 400)
    return pka, pkb


def _combine(pvecs):
    """Final cross-core reduction + divides, mirroring the reference math."""
    P = np.zeros(8, dtype=np.float32)
    for v in pvecs:
        P = P + v.astype(np.float32)
    heat = P[1] / max(P[0], np.float32(1.0))
    if P[2] > 1.0:
        box = P[3] / max(P[2], np.float32(1.0))
        cls = (P[4] + P[6]) / max(P[5], np.float32(1.0))
    else:
        box = np.float32(0.0)
        cls = np.float32(0.0)
    return np.float32(heat + box + cls)


def kernel(pred_heatmap, pred_boxes, pred_classes, bboxes, labels):
    if "nc" not in _CACHE:
        _CACHE["nc"] = _build()
    nc = _CACHE["nc"]

    qg2, utri, cvec, chm = _consts()
    pbt = np.ascontiguousarray(pred_boxes.transpose(0, 2, 3, 1).reshape(B, PIX, 4))
    pct = np.ascontiguousarray(pred_classes.transpose(0, 2, 3, 1).reshape(B, PIX, NCLS))
    hmf = np.ascontiguousarray(pred_heatmap.reshape(B, PIX)).astype(np.float32)
    lab32 = np.asarray(labels).astype(np.int32)

    in_maps = []
    for c in range(NC):
        s = slice(c * BPC, (c + 1) * BPC)
        pka, pkb = _pack(np.asarray(bboxes[s], dtype=np.float32), lab32[s], hmf[s], utri, cvec, chm)
        in_maps.append({
            "pka": pka, "pkb": pkb, "q2": qg2,
            "pbt": pbt[s].reshape(BPC * PIX, 4),
            "pct": pct[s].reshape(BPC * PIX, NCLS),
        })

    r = run_bass_kernel_spmd(nc, in_maps, list(range(NC)))
    return _combine([np.asarray(r.results[c]["out"]).reshape(8) for c in range(NC)])


if __name__ == "__main__":
    import reference
    inputs = reference.setup_inputs()
    inputs = {k: np.asarray(v) for k, v in inputs.items()}
    out = kernel(**inputs)
    exp = np.asarray(reference.reference(**{k: v for k, v in inputs.items()}))
    rel = abs(out - exp) / max(abs(exp), 1e-9)
    print("expected:", exp, "actual:", out, "rel:", rel)


# revision 3
# speedup vs baseline: 1.0533x; 1.0126x over previous
"""AnchorFreeLoss on 8 TRN2 NeuronCores — v7.

On top of v3 (fp32r 512-wide matmuls, packed DMA, paired PSUM reduces,
host-side final combine):
- pk split: stage-A columns [128,56] land ~4us before the bulk (utri/hmP).
- DVE program reordered: pair reduces begin as soon as the PE fills the
  first pair; dedup/l1/cls blocks slot between early reduces instead of
  blocking the reduce stream.
- Focal restructured: p-derived planes A=-0.25(1-p)^2 ln p and
  B'=0.75 p^2 ln(1-p) are computed from the input heatmap before the
  reduce stream; pos is computed in log domain (no exp dependency); the
  pos-weighted sum uses stt accum_out. Tail after the last reduce is
  ~6 DVE ops + one scalar exp.
- Optional fp16 staging (USE_F16): gpsimd cast-DMAs each PSUM pair to
  fp16 SBUF; DVE reduces from SBUF at the 2x/4x DVE rate.
"""

import sys
from contextlib import ExitStack

import numpy as np

if "/opt/trn_rl_repo" not in sys.path:
    sys.path.insert(0, "/opt/trn_rl_repo")

from concourse import bass, mybir
from concourse.bass_utils import run_bass_kernel_spmd

F32 = mybir.dt.float32
F32R = mybir.dt.float32r
F16 = mybir.dt.float16
I32 = mybir.dt.int32
ALU = mybir.AluOpType
ACT = mybir.ActivationFunctionType

B, M, H, W = 16, 64, 160, 160
NC = 8
BPC = B // NC
PIX = H * W
NCLS = 43
EPS = 1e-7
LN4 = 1.3862943611198906
LNH = -0.6931471805599453   # ln(0.5)
THR = -8.0
NBANK = 50
NPAIR = 25
PDEPTH = 3
CTR = 80.0

USE_F16 = False

# pkA columns (stage-A critical)
PK_BB = 0
PK_LAB = 4
PK_CV = 5
PK_CHM = 13
PKA_N = 56
# pkB columns (bulk)
PKB_UT = 0
PKB_HM = 128
PKB_N = 528


def _build(debug=False):
    nc = bass.Bass()

    pka_d = nc.declare_dram_parameter("pka", [128, PKA_N], F32, isOutput=False)
    pkb_d = nc.declare_dram_parameter("pkb", [128, PKB_N], F32, isOutput=False)
    q2_d = nc.declare_dram_parameter("q2", [16, 6912], F32R, isOutput=False)
    pb_d = nc.declare_dram_parameter("pbt", [BPC * PIX, 4], F32, isOutput=False)
    pc_d = nc.declare_dram_parameter("pct", [BPC * PIX, NCLS], F32, isOutput=False)
    out_d = nc.declare_dram_parameter("out", [1, 8], F32, isOutput=True)
    dbg = {}
    if debug:
        for nm, shp in [("d_partials", [128, 8]), ("d_sc", [128, 48]),
                        ("d_hmL", [128, 400]), ("d_W5", [128, 32])]:
            dbg[nm] = nc.declare_dram_parameter(nm, shp, F32, isOutput=True)

    dbc = nc.dram_tensor("dbc", [2, 128], F32)

    es = ExitStack()
    dma_in = es.enter_context(nc.semaphore("dma_in"))
    dma_pk = es.enter_context(nc.semaphore("dma_pk"))
    dma_pb = es.enter_context(nc.semaphore("dma_pb"))
    dma2 = es.enter_context(nc.semaphore("dma2"))
    d6 = es.enter_context(nc.semaphore("d6"))
    va = es.enter_context(nc.semaphore("va"))
    vf = es.enter_context(nc.semaphore("vf"))
    av = es.enter_context(nc.semaphore("av"))
    wsem = es.enter_context(nc.semaphore("wsem"))
    tr_s = es.enter_context(nc.semaphore("tr_s"))
    tr2_s = es.enter_context(nc.semaphore("tr2_s"))
    pe_s = es.enter_context(nc.semaphore("pe_s"))
    dv_s = es.enter_context(nc.semaphore("dv_s"))
    st_s = es.enter_context(nc.semaphore("st_s"))
    cell_s = es.enter_context(nc.semaphore("cell_s"))
    g_s = es.enter_context(nc.semaphore("g_s"))
    pt_s = es.enter_context(nc.semaphore("pt_s"))
    pp_s = es.enter_context(nc.semaphore("pp_s"))
    pv_s = es.enter_context(nc.semaphore("pv_s"))
    pka = es.enter_context(nc.sbuf_tensor("pka_s", [128, PKA_N], F32))
    pkb = es.enter_context(nc.sbuf_tensor("pkb_s", [128, PKB_N], F32))
    sQ2 = es.enter_context(nc.sbuf_tensor("sQ2", [16, 6400], F32R))
    blkW = es.enter_context(nc.sbuf_tensor("blkW", [16, 512], F32R))
    W5 = es.enter_context(nc.sbuf_tensor("W5", [128, 32], F32))
    tmpT = es.enter_context(nc.sbuf_tensor("tmpT", [32, 128], F32))
    tmpT2 = es.enter_context(nc.sbuf_tensor("tmpT2", [32, 128], F32))
    sc = es.enter_context(nc.sbuf_tensor("sc", [128, 48], F32))
    sci = es.enter_context(nc.sbuf_tensor("sci", [128, 4], I32))
    if USE_F16:
        hmL = es.enter_context(nc.sbuf_tensor("hmL", [128, 400], F16))
        stg = es.enter_context(nc.sbuf_tensor("stg", [128, PDEPTH * 1024], F16))
    else:
        hmL = es.enter_context(nc.sbuf_tensor("hmL", [128, 400], F32))
        stg = None
    fw0 = es.enter_context(nc.sbuf_tensor("fw0", [128, 400], F32))
    fw1 = es.enter_context(nc.sbuf_tensor("fw1", [128, 400], F32))
    fw2 = es.enter_context(nc.sbuf_tensor("fw2", [128, 400], F32))
    fw3 = es.enter_context(nc.sbuf_tensor("fw3", [128, 400], F32))
    fw4 = es.enter_context(nc.sbuf_tensor("fw4", [128, 400], F32))
    fw5 = es.enter_context(nc.sbuf_tensor("fw5", [128, 400], F32))
    fw6 = es.enter_context(nc.sbuf_tensor("fw6", [128, 400], F32))
    cbc = es.enter_context(nc.sbuf_tensor("cbc", [128, 128], F32))
    kbc = es.enter_context(nc.sbuf_tensor("kbc", [128, 128], F32))
    eqt = es.enter_context(nc.sbuf_tensor("eqt", [128, 128], F32))
    junkm = es.enter_context(nc.sbuf_tensor("junkm", [128, 128], F32))
    partials = es.enter_context(nc.sbuf_tensor("partials", [128, 8], F32))
    gb = es.enter_context(nc.sbuf_tensor("gb", [128, 4], F32))
    gc = es.enter_context(nc.sbuf_tensor("gc", [128, NCLS], F32))
    gcp = es.enter_context(nc.sbuf_tensor("gcp", [128, NCLS], F32))
    junk43 = es.enter_context(nc.sbuf_tensor("junk43", [128, NCLS], F32))
    junk4 = es.enter_context(nc.sbuf_tensor("junk4", [128, 4], F32))
    tbox = es.enter_context(nc.sbuf_tensor("tbox", [128, 4], F32))
    pvec = es.enter_context(nc.sbuf_tensor("pvec", [1, 8], F32))
    pp0 = es.enter_context(nc.psum_tensor("pp0", [128, 2048], F32))
    pp1 = es.enter_context(nc.psum_tensor("pp1", [128, 2048], F32))
    with es:
        pp = [pp0, pp1]
        psp = pp0[0:1, 0:8]

        names = [
            "sumx", "sumy", "csx", "csy", "gxf", "gyf", "bw", "bh", "bbs",
            "vbw", "vbh", "vbs", "vlab", "valid", "rq", "rm", "rf", "r2",
            "rr", "gxc", "gyc", "gx2", "gy2", "g2s", "w3a", "pen",
            "t2a", "t2b", "cellf", "cellgf", "labcf", "keyf",
            "cva", "sent", "kept", "keep2", "later1", "later2",
            "l1r", "negrow", "plab", "lnp", "ln1mp", "psq", "mpsq", "sumy2",
        ]
        col = {n: sc[:, i: i + 1] for i, n in enumerate(names)}

        idxp1 = pka[:, PK_CV + 0: PK_CV + 1]
        ones = pka[:, PK_CV + 1: PK_CV + 2]
        basef = pka[:, PK_CV + 2: PK_CV + 3]
        nc.const_aps.aps[(F32, 0.0)] = pka[:, PK_CV + 3: PK_CV + 4]
        nc.const_aps.aps[(F32, 1.0)] = ones
        nc.const_aps.aps[(F32, 1e-6)] = pka[:, PK_CV + 4: PK_CV + 5]
        nc.const_aps.aps[(F32, -LN4)] = pka[:, PK_CV + 5: PK_CV + 6]
        labf = pka[:, PK_LAB: PK_LAB + 1]
        chm = pka[:, PK_CHM: PK_CHM + NCLS]
        utri = pkb[:, PKB_UT: PKB_UT + 128]
        pkhm = pkb[:, PKB_HM: PKB_HM + 400]
        cellg = sci[:, 2:3]

        with nc.Block() as block:

            @block.sync
            def _(sync):
                sync.dma_start(out=pka[:, :], in_=pka_d[:, :]).then_inc(dma_pk, 16)
                sync.dma_start(out=sQ2[:, :], in_=q2_d[:, 0:6400]).then_inc(dma_in, 16)
                sync.dma_start(out=blkW[:, :], in_=q2_d[:, 6400:6912]).then_inc(dma_in, 16)
                sync.dma_start(out=pkb[:, :], in_=pkb_d[:, :]).then_inc(dma_pb, 16)
                sync.wait_ge(tr2_s, 1)
                sync.dma_start(out=dbc[0:1, :], in_=tmpT2[4:5, :]).then_inc(dma2, 16)
                sync.dma_start(out=dbc[1:2, :], in_=tmpT2[5:6, :]).then_inc(dma2, 16)
                sync.wait_ge(dma2, 32)
                sync.dma_start(out=cbc[:, :], in_=dbc[0:1, :].to_broadcast([128, 128])).then_inc(dma2, 16)
                sync.dma_start(out=kbc[:, :], in_=dbc[1:2, :].to_broadcast([128, 128])).then_inc(dma2, 16)
                # partial-sum vector out (host combines across cores)
                sync.wait_ge(pv_s, 1)
                sync.dma_start(out=out_d[:, :], in_=pvec[:, :]).then_inc(d6, 16)
                nd6 = 16
                if debug:
                    for nm, t in [("d_partials", partials), ("d_sc", sc),
                                  ("d_hmL", hmL), ("d_W5", W5)]:
                        sync.dma_start(out=dbg[nm][:, :], in_=t[:, :]).then_inc(d6, 16)
                        nd6 += 16
                sync.wait_ge(d6, nd6)

            @block.scalar
            def _(scalar):
                scalar.wait_ge(va, 1)
                scalar.activation(col["rq"], col["bbs"], ACT.Ln)
                scalar.drain()
                scalar.activation(col["rq"], col["rq"], ACT.Exp, bias=-LN4, scale=0.5)
                scalar.activation(tbox[:, 2:4], sc[:, 6:8], ACT.Ln, bias=1e-6, scale=0.25)
                scalar.drain()
                scalar.sem_inc(av, 1)
                # early focal transcendentals from p (input-only)
                scalar.wait_ge(vf, 1)
                scalar.activation(fw2[:, :], fw6[:, :], ACT.Ln)
                scalar.activation(fw3[:, :], fw6[:, :], ACT.Ln, bias=1.0, scale=-1.0)
                scalar.activation(fw4[:, :], fw6[:, :], ACT.Square)
                scalar.activation(fw5[:, :], fw6[:, :], ACT.Square, bias=1.0, scale=-1.0)
                scalar.drain()
                scalar.sem_inc(av, 1)
                # cls sigmoid via exp
                scalar.wait_ge(g_s, 32)
                scalar.activation(gcp[:, :], gc[:, :], ACT.Exp, scale=-1.0)
                scalar.drain()
                scalar.sem_inc(av, 1)
                scalar.wait_ge(va, 2)
                scalar.activation(junk43[:, :], gcp[:, :], ACT.Ln, bias=1.0, scale=-1.0)
                scalar.activation(gc[:, :], gcp[:, :], ACT.Square)
                scalar.drain()
                scalar.sem_inc(av, 1)
                scalar.wait_ge(va, 3)
                scalar.activation(col["lnp"], col["plab"], ACT.Ln)
                scalar.activation(col["ln1mp"], col["plab"], ACT.Ln, bias=1.0, scale=-1.0)
                scalar.activation(col["psq"], col["plab"], ACT.Square)
                scalar.activation(col["mpsq"], col["plab"], ACT.Square, bias=1.0, scale=-1.0)
                scalar.drain()
                scalar.sem_inc(av, 1)
                # t = exp(clamped log heatmap)
                scalar.wait_ge(va, 4)
                scalar.activation(fw1[:, :], fw0[:, :], ACT.Exp)
                scalar.drain()
                scalar.sem_inc(av, 1)

            @block.tensor
            def _(tensor):
                tensor.wait_ge(wsem, 64)
                tensor.wait_ge(dma_in, 32)
                for g in range(NBANK):
                    q = min(g // 4, 12)
                    pt = pp[q % 2]
                    off = (g % 4) * 512 if g < 48 else (g - 48) * 512
                    if q >= 2 and g % 4 == 0 or g == 48:
                        tensor.wait_ge(dv_s, q - 1)
                    tensor.matmul(
                        pt[:, off: off + 512],
                        sQ2[:, g * 128: (g + 1) * 128],
                        blkW[:, :],
                        start=True,
                        stop=True,
                        skip_group_check=True,
                    ).then_inc(pe_s, 1)
                tensor.wait_ge(pt_s, 1)
                tensor.matmul(psp, ones, partials[:, :], start=True, stop=True, skip_group_check=True).then_inc(pp_s, 1)

            @block.gpsimd
            def _(gpsimd):
                gpsimd.wait_ge(dma_in, 32)
                gpsimd.wait_ge(tr_s, 1)
                for c in range(4):
                    gpsimd.dma_start(
                        out=blkW[4 * c: 4 * c + 4, c * 128: (c + 1) * 128],
                        in_=tmpT[0:4, :],
                    ).then_inc(wsem, 16)
                gpsimd.wait_ge(cell_s, 1)
                gpsimd.indirect_dma_start(
                    out=gb[:, :], out_offset=None,
                    in_=pb_d[:, :],
                    in_offset=bass.IndirectOffsetOnAxis(ap=cellg, axis=0),
                ).then_inc(g_s, 16)
                gpsimd.indirect_dma_start(
                    out=gc[:, :], out_offset=None,
                    in_=pc_d[:, :],
                    in_offset=bass.IndirectOffsetOnAxis(ap=cellg, axis=0),
                ).then_inc(g_s, 16)
                if USE_F16:
                    # fp16 cast staging of each PSUM pair (frees the bank for PE)
                    for gp in range(NPAIR):
                        gpsimd.wait_ge(pe_s, 2 * (gp + 1))
                        gpsimd.dma_start(
                            out=stg[:, (gp % PDEPTH) * 1024: (gp % PDEPTH) * 1024 + 1024],
                            in_=pp[gp % PDEPTH][:, :],
                        ).then_inc(st_s, 16)

            @block.vector
            def _(v):
                ts, stt = v.tensor_scalar, v.scalar_tensor_tensor

                def D():
                    v.drain()

                v.wait_ge(dma_pk, 16)
                # ---- stage A (area first: scalar round-trip starts asap) ----
                v.tensor_sub(sc[:, 6:8], pka[:, 2:4], pka[:, 0:2])
                D()
                v.tensor_mul(col["bbs"], col["bw"], col["bh"])
                D()
                v.sem_inc(va, 1)
                v.tensor_add(sc[:, 0:2], pka[:, 0:2], pka[:, 2:4])
                D()
                ts(sc[:, 2:4], sc[:, 0:2], 0.125, 0.5, op0=ALU.mult, op1=ALU.subtract)
                D()
                v.tensor_copy(sci[:, 0:2], sc[:, 2:4])
                D()
                v.tensor_copy(sc[:, 4:6], sci[:, 0:2])
                D()
                ts(sc[:, 4:6], sc[:, 4:6], 0.0, 159.0, op0=ALU.max, op1=ALU.min)
                D()
                v.tensor_reduce(out=col["rm"], in_=pka[:, 0:4], op=ALU.add, axis=mybir.AxisListType.X)
                D()
                ts(sc[:, 9:11], sc[:, 6:8], 0.0, None, op0=ALU.is_gt)
                D()
                ts(col["vbs"], col["rm"], 0.0, None, op0=ALU.is_gt)
                D()
                ts(col["vlab"], labf, 0.0, None, op0=ALU.is_ge)
                D()
                v.tensor_reduce(out=col["valid"], in_=sc[:, 9:13], op=ALU.mult, axis=mybir.AxisListType.X)
                D()
                ts(sc[:, 19:21], sc[:, 4:6], CTR, None, op0=ALU.subtract)
                D()
                v.tensor_mul(sc[:, 21:23], sc[:, 19:21], sc[:, 19:21])
                D()
                v.tensor_add(col["g2s"], col["gx2"], col["gy2"])
                D()
                ts(col["pen"], col["valid"], 1.0, 1e30, op0=ALU.subtract, op1=ALU.mult)
                D()
                v.wait_ge(av, 1)
                ts(col["rm"], col["rq"], 2.0, 0.5, op0=ALU.max, op1=ALU.subtract)
                D()
                v.tensor_copy(sci[:, 3:4], col["rm"])
                D()
                v.tensor_copy(col["rf"], sci[:, 3:4])
                D()
                v.tensor_mul(col["r2"], col["rf"], col["rf"])
                D()
                v.reciprocal(col["rr"], col["r2"])
                D()
                ts(W5[:, 0:1], col["rr"], -2.0, None, op0=ALU.mult)
                D()
                v.tensor_mul(col["w3a"], W5[:, 0:1], col["g2s"])
                D()
                v.tensor_add(W5[:, 3:4], col["w3a"], col["pen"])
                D()
                ts(sc[:, 26:28], sc[:, 19:21], W5[:, 0:1], None, op0=ALU.mult)
                D()
                ts(W5[:, 1:3], sc[:, 26:28], -2.0, None, op0=ALU.mult)
                D()
                for c4 in range(4):
                    v.transpose(tmpT[0:32, c4 * 32: (c4 + 1) * 32], W5[c4 * 32: (c4 + 1) * 32, 0:32])
                D()
                v.sem_inc(tr_s, 1)
                # cell/key
                stt(col["cellf"], col["gyf"], 160.0, col["gxf"], op0=ALU.mult, op1=ALU.add)
                D()
                v.tensor_add(col["cellgf"], col["cellf"], basef)
                D()
                v.tensor_copy(cellg, col["cellgf"])
                D()
                v.sem_inc(cell_s, 1)
                ts(col["labcf"], labf, 0.0, 42.0, op0=ALU.max, op1=ALU.min)
                D()
                stt(col["keyf"], col["cellgf"], 43.0, col["labcf"], op0=ALU.mult, op1=ALU.add)
                D()
                v.tensor_mul(col["cva"], col["cellgf"], col["valid"])
                D()
                stt(col["sent"], col["valid"], 1.0, idxp1, op0=ALU.subtract, op1=ALU.mult)
                D()
                v.tensor_add(W5[:, 4:5], col["cva"], col["sent"])
                D()
                v.tensor_mul(col["cva"], col["keyf"], col["valid"])
                D()
                v.tensor_add(W5[:, 5:6], col["cva"], col["sent"])
                D()
                for c4 in range(4):
                    v.transpose(tmpT2[0:32, c4 * 32: (c4 + 1) * 32], W5[c4 * 32: (c4 + 1) * 32, 0:32])
                D()
                v.sem_inc(tr2_s, 1)
                v.tensor_sub(tbox[:, 0:2], sc[:, 2:4], sc[:, 4:6])
                D()
                # ---- focal prelude from input heatmap ----
                v.wait_ge(dma_pb, 16)
                ts(fw6[:, :], pkhm, EPS, 1.0 - EPS, op0=ALU.max, op1=ALU.min)  # p
                D()
                v.sem_inc(vf, 1)      # scalar: fw2..fw5
                # ---- dedup ----
                v.wait_ge(dma2, 64)
                ts(eqt[:, :], cbc[:, :], W5[:, 4:5], None, op0=ALU.is_equal)
                D()
                v.tensor_mul(junkm[:, :], eqt[:, :], utri)
                D()
                v.tensor_reduce(out=col["later1"], in_=junkm[:, :], op=ALU.max, axis=mybir.AxisListType.X)
                D()
                ts(col["cva"], col["later1"], -1.0, 1.0, op0=ALU.mult, op1=ALU.add)
                D()
                v.tensor_mul(partials[:, 2:3], col["valid"], col["cva"])
                D()
                v.tensor_copy(col["kept"], partials[:, 2:3])
                D()
                ts(eqt[:, :], kbc[:, :], W5[:, 5:6], None, op0=ALU.is_equal)
                D()
                v.tensor_mul(junkm[:, :], eqt[:, :], utri)
                D()
                v.tensor_reduce(out=col["later2"], in_=junkm[:, :], op=ALU.max, axis=mybir.AxisListType.X)
                D()
                ts(col["cva"], col["later2"], -1.0, 1.0, op0=ALU.mult, op1=ALU.add)
                D()
                v.tensor_mul(partials[:, 5:6], col["valid"], col["cva"])
                D()
                v.tensor_copy(col["keep2"], partials[:, 5:6])
                D()

                def reduce_quad(q):
                    if q < 12:
                        v.wait_ge(pe_s, 4 * (q + 1))
                        src = pp[q % 2][:, :]
                        na, lo = 16, 16 * q
                    else:
                        v.wait_ge(pe_s, 50)
                        src = pp[0][:, 0:1024]
                        na, lo = 8, 192
                    v.tensor_reduce(
                        out=hmL[:, :].rearrange("p (i f) -> p f i", i=2)[:, lo: lo + na, :],
                        in_=src.rearrange("p (a b m) -> p a b m", a=na, b=2, m=64),
                        op=ALU.max,
                        axis=mybir.AxisListType.X,
                    )
                    v.drain().then_inc(dv_s, 1)

                nxt = 0
                reduce_quad(nxt); nxt += 1
                # ---- box l1 ----
                v.wait_ge(g_s, 32)
                v.tensor_sub(junk4[:, :], gb[:, :], tbox[:, :])
                D()
                ts(gb[:, :], junk4[:, :], -1.0, None, op0=ALU.mult)
                D()
                v.tensor_tensor(junk4[:, :], junk4[:, :], gb[:, :], op=ALU.max)
                D()
                v.tensor_reduce(out=col["l1r"], in_=junk4[:, :], op=ALU.add, axis=mybir.AxisListType.X)
                D()
                v.tensor_mul(partials[:, 3:4], col["l1r"], col["kept"])
                D()
                reduce_quad(nxt); nxt += 1
                # ---- cls neg ----
                v.wait_ge(av, 3)
                ts(gcp[:, :], gcp[:, :], 1.0, None, op0=ALU.add)
                D()
                v.reciprocal(gcp[:, :], gcp[:, :])
                D()
                ts(gcp[:, :], gcp[:, :], EPS, 1.0 - EPS, op0=ALU.max, op1=ALU.min)
                D()
                v.sem_inc(va, 1)
                reduce_quad(nxt); nxt += 1
                v.wait_ge(av, 4)
                stt(junk43[:, :], gc[:, :], -0.75, junk43[:, :], op0=ALU.mult, op1=ALU.mult, accum_out=col["negrow"])
                D()
                v.tensor_mul(partials[:, 4:5], col["negrow"], col["kept"])
                D()
                # ---- cls pos ----
                ts(eqt[:, 0:NCLS], chm, col["labcf"], None, op0=ALU.is_equal)
                D()
                v.tensor_mul(junk43[:, :], gcp[:, :], eqt[:, 0:NCLS])
                D()
                v.tensor_reduce(out=col["plab"], in_=junk43[:, :], op=ALU.add, axis=mybir.AxisListType.X)
                D()
                v.sem_inc(va, 1)
                reduce_quad(nxt); nxt += 1
                v.wait_ge(av, 5)
                stt(col["cva"], col["mpsq"], -0.25, col["lnp"], op0=ALU.mult, op1=ALU.mult)
                D()
                stt(col["sent"], col["psq"], -0.75, col["ln1mp"], op0=ALU.mult, op1=ALU.mult)
                D()
                v.tensor_sub(col["cva"], col["cva"], col["sent"])
                D()
                v.tensor_mul(partials[:, 6:7], col["cva"], col["keep2"])
                D()
                v.memset(partials[:, 7:8], 0.0)
                D()
                # focal planes A (fw2) and B' (fw3) — input-only, hide before reduces
                v.wait_ge(av, 2)
                stt(fw2[:, :], fw5[:, :], -0.25, fw2[:, :], op0=ALU.mult, op1=ALU.mult)  # A
                D()
                stt(fw3[:, :], fw4[:, :], 0.75, fw3[:, :], op0=ALU.mult, op1=ALU.mult)   # B'
                D()
                # ---- remaining reduce quads ----
                while nxt < 13:
                    reduce_quad(nxt); nxt += 1
                # ---- heat focal tail ----
                ts(fw0[:, :], hmL[:, :], -80.0, None, op0=ALU.max)
                D()
                v.sem_inc(va, 1)                    # scalar: fw1 = exp(fw0)
                ts(fw4[:, :], fw0[:, :], THR, None, op0=ALU.is_ge)       # keep mask
                D()
                # pos in log domain: (fw0 > ln 0.5) * mask
                stt(fw5[:, :], fw0[:, :], LNH, fw4[:, :], op0=ALU.is_gt, op1=ALU.mult)
                D()
                v.tensor_reduce(out=partials[:, 0:1], in_=fw5[:, :], op=ALU.add, axis=mybir.AxisListType.X)
                D()
                v.wait_ge(av, 6)
                v.tensor_mul(fw1[:, :], fw1[:, :], fw4[:, :])            # t
                D()
                v.tensor_mul(fw4[:, :], fw2[:, :], fw1[:, :])            # X = A*t
                D()
                stt(fw6[:, :], fw1[:, :], 1.0, fw3[:, :], op0=ALU.subtract, op1=ALU.mult,
                    accum_out=col["sumy2"])                              # Y, sum(Y)
                D()
                v.tensor_sub(fw4[:, :], fw4[:, :], fw6[:, :])            # X - Y
                D()
                stt(fw4[:, :], fw4[:, :], 1.0, fw5[:, :], op0=ALU.mult, op1=ALU.mult,
                    accum_out=col["cva"])                                # (X-Y)*pos, sum
                D()
                v.tensor_add(partials[:, 1:2], col["cva"], col["sumy2"])
                D()
                v.sem_inc(pt_s, 1)
                v.wait_ge(pp_s, 1)
                v.tensor_copy(pvec[:, :], psp)
                D()
                v.sem_inc(pv_s, 1)

    return nc


_CACHE = {}


def _consts():
    p = np.arange(128)
    g = np.arange(NBANK)
    qg2 = np.zeros((16, 6912), np.float32)
    for c in range(4):
        pix = p[None, :] * 200 + 4 * g[:, None] + c
        xx = (pix % W).astype(np.float32) - CTR
        yy = (pix // W).astype(np.float32) - CTR
        q4 = np.stack([xx * xx + yy * yy, xx, yy, np.ones_like(xx)])
        qg2[4 * c: 4 * c + 4, 0:6400] = q4.reshape(4, 6400)
    utri = np.triu(np.ones((128, 128), dtype=np.float32), k=1)
    cvec = np.zeros((128, 8), dtype=np.float32)
    cvec[:, 0] = np.arange(128) + 1.0
    cvec[:, 1] = 1.0
    cvec[64:, 2] = PIX
    cvec[:, 4] = 1e-6
    cvec[:, 5] = -LN4
    chm = np.broadcast_to(np.arange(NCLS, dtype=np.float32), (128, NCLS))
    return qg2, utri, cvec, chm


def _pack(bb, lab32, hmf, utri, cvec, chm):
    pka = np.zeros((128, PKA_N), dtype=np.float32)
    pka[:, PK_BB: PK_BB + 4] = bb.reshape(128, 4)
    pka[:, PK_LAB] = lab32.reshape(128).astype(np.float32)
    pka[:, PK_CV: PK_CV + 8] = cvec
    pka[:, PK_CHM: PK_CHM + NCLS] = chm
    pkb = np.zeros((128, PKB_N), dtype=np.float32)
    pkb[:, PKB_UT: PKB_UT + 128] = utri
    pkb[:, PKB_HM: PKB_HM + 400] = hmf.reshape(BPC, 128, 200).transpose(1, 0, 2).reshape(128, 400)
    return pka, pkb


def _combine(pvecs):
    """Final cross-core reduction + divides, mirroring the reference math."""
    P = np.zeros(8, dtype=np.float32)
    for v in pvecs:
        P = P + v.astype(np.float32)
    heat = P[1] / max(P[0], np.float32(1.0))
    if P[2] > 1.0:
        box = P[3] / max(P[2], np.float32(1.0))
        cls = (P[4] + P[6]) / max(P[5], np.float32(1.0))
    else:
        box = np.float32(0.0)
        cls = np.float32(0.0)
    return np.float32(heat + box + cls)


def kernel(pred_heatmap, pred_boxes, pred_classes, bboxes, labels):
    if "nc" not in _CACHE:
        _CACHE["nc"] = _build()
    nc = _CACHE["nc"]

    qg2, utri, cvec, chm = _consts()
    pbt = np.ascontiguousarray(pred_boxes.transpose(0, 2, 3, 1).reshape(B, PIX, 4))
    pct = np.ascontiguousarray(pred_classes.transpose(0, 2, 3, 1).reshape(B, PIX, NCLS))
    hmf = np.ascontiguousarray(pred_heatmap.reshape(B, PIX)).astype(np.float32)
    lab32 = np.asarray(labels).astype(np.int32)

    in_maps = []
    for c in range(NC):
        s = slice(c * BPC, (c + 1) * BPC)
        pka, pkb = _pack(np.asarray(bboxes[s], dtype=np.float32), lab32[s], hmf[s], utri, cvec, chm)
        in_maps.append({
            "pka": pka, "pkb": pkb, "q2": qg2,
            "pbt": pbt[s].reshape(BPC * PIX, 4),
            "pct": pct[s].reshape(BPC * PIX, NCLS),
        })

    r = run_bass_kernel_spmd(nc, in_maps, list(range(NC)))
    return _combine([np.asarray(r.results[c]["out"]).reshape(8) for c in range(NC)])


if __name__ == "__main__":
    import reference
    inputs = reference.setup_inputs()
    inputs = {k: np.asarray(v) for k, v in inputs.items()}
    out = kernel(**inputs)
    exp = np.asarray(reference.reference(**{k: v for k, v in inputs.items()}))
    rel = abs(out - exp) / max(abs(exp), 1e-9)
    print("expected:", exp, "actual:", out, "rel:", rel)


# revision 4
# speedup vs baseline: 1.1003x; 1.0447x over previous
"""AnchorFreeLoss on 8 TRN2 NeuronCores — v8.

On top of v3 (fp32r 512-wide matmuls, packed DMA, paired PSUM reduces,
host-side final combine):
- pk split: stage-A columns [128,56] land ~4us before the bulk (utri/hmP).
- DVE program reordered: pair reduces begin as soon as the PE fills the
  first pair; dedup/l1/cls blocks slot between early reduces instead of
  blocking the reduce stream.
- Focal restructured: p-derived planes A=-0.25(1-p)^2 ln p and
  B'=0.75 p^2 ln(1-p) are computed from the input heatmap before the
  reduce stream; pos is computed in log domain (no exp dependency); the
  pos-weighted sum uses stt accum_out. Tail after the last reduce is
  ~6 DVE ops + one scalar exp.
- Optional fp16 staging (USE_F16): gpsimd cast-DMAs each PSUM pair to
  fp16 SBUF; DVE reduces from SBUF at the 2x/4x DVE rate.
"""

import sys
from contextlib import ExitStack

import numpy as np

if "/opt/trn_rl_repo" not in sys.path:
    sys.path.insert(0, "/opt/trn_rl_repo")

from concourse import bass, mybir
from concourse.bass_utils import run_bass_kernel_spmd

F32 = mybir.dt.float32
F32R = mybir.dt.float32r
F16 = mybir.dt.float16
I32 = mybir.dt.int32
ALU = mybir.AluOpType
ACT = mybir.ActivationFunctionType

B, M, H, W = 16, 64, 160, 160
NC = 8
BPC = B // NC
PIX = H * W
NCLS = 43
EPS = 1e-7
LN4 = 1.3862943611198906
LNH = -0.6931471805599453   # ln(0.5)
THR = -8.0
NBANK = 50
NPAIR = 25
PDEPTH = 3
CTR = 80.0

USE_F16 = False

# pkA columns (stage-A critical)
PK_BB = 0
PK_LAB = 4
PK_CV = 5
PK_CHM = 13
PKA_N = 56
# pkB columns (bulk)
PKB_UT = 0
PKB_HM = 128
PKB_N = 528


def _build(debug=False):
    nc = bass.Bass()

    pka_d = nc.declare_dram_parameter("pka", [128, PKA_N], F32, isOutput=False)
    pkb_d = nc.declare_dram_parameter("pkb", [128, PKB_N], F32, isOutput=False)
    q2_d = nc.declare_dram_parameter("q2", [16, 6912], F32R, isOutput=False)
    pb_d = nc.declare_dram_parameter("pbt", [BPC * PIX, 4], F32, isOutput=False)
    pc_d = nc.declare_dram_parameter("pct", [BPC * PIX, NCLS], F32, isOutput=False)
    out_d = nc.declare_dram_parameter("out", [1, 8], F32, isOutput=True)
    dbg = {}
    if debug:
        for nm, shp in [("d_partials", [128, 8]), ("d_sc", [128, 48]),
                        ("d_hmL", [128, 400]), ("d_W5", [128, 32])]:
            dbg[nm] = nc.declare_dram_parameter(nm, shp, F32, isOutput=True)

    dbc = nc.dram_tensor("dbc", [2, 128], F32)

    es = ExitStack()
    dma_in = es.enter_context(nc.semaphore("dma_in"))
    dma_pk = es.enter_context(nc.semaphore("dma_pk"))
    dma_pb = es.enter_context(nc.semaphore("dma_pb"))
    dma2 = es.enter_context(nc.semaphore("dma2"))
    d6 = es.enter_context(nc.semaphore("d6"))
    va = es.enter_context(nc.semaphore("va"))
    vf = es.enter_context(nc.semaphore("vf"))
    av = es.enter_context(nc.semaphore("av"))
    wsem = es.enter_context(nc.semaphore("wsem"))
    tr_s = es.enter_context(nc.semaphore("tr_s"))
    tr2_s = es.enter_context(nc.semaphore("tr2_s"))
    pe_s = es.enter_context(nc.semaphore("pe_s"))
    dv_s = es.enter_context(nc.semaphore("dv_s"))
    st_s = es.enter_context(nc.semaphore("st_s"))
    cell_s = es.enter_context(nc.semaphore("cell_s"))
    g_s = es.enter_context(nc.semaphore("g_s"))
    pt_s = es.enter_context(nc.semaphore("pt_s"))
    pp_s = es.enter_context(nc.semaphore("pp_s"))
    pv_s = es.enter_context(nc.semaphore("pv_s"))
    pka = es.enter_context(nc.sbuf_tensor("pka_s", [128, PKA_N], F32))
    pkb = es.enter_context(nc.sbuf_tensor("pkb_s", [128, PKB_N], F32))
    sQ2 = es.enter_context(nc.sbuf_tensor("sQ2", [16, 6400], F32R))
    blkW = es.enter_context(nc.sbuf_tensor("blkW", [16, 512], F32R))
    W5 = es.enter_context(nc.sbuf_tensor("W5", [128, 32], F32))
    tmpT = es.enter_context(nc.sbuf_tensor("tmpT", [32, 128], F32))
    tmpT2 = es.enter_context(nc.sbuf_tensor("tmpT2", [32, 128], F32))
    sc = es.enter_context(nc.sbuf_tensor("sc", [128, 48], F32))
    sci = es.enter_context(nc.sbuf_tensor("sci", [128, 4], I32))
    if USE_F16:
        hmL = es.enter_context(nc.sbuf_tensor("hmL", [128, 400], F16))
        stg = es.enter_context(nc.sbuf_tensor("stg", [128, PDEPTH * 1024], F16))
    else:
        hmL = es.enter_context(nc.sbuf_tensor("hmL", [128, 400], F32))
        stg = None
    fw0 = es.enter_context(nc.sbuf_tensor("fw0", [128, 400], F32))
    fw1 = es.enter_context(nc.sbuf_tensor("fw1", [128, 400], F32))
    fw2 = es.enter_context(nc.sbuf_tensor("fw2", [128, 400], F32))
    fw3 = es.enter_context(nc.sbuf_tensor("fw3", [128, 400], F32))
    fw4 = es.enter_context(nc.sbuf_tensor("fw4", [128, 400], F32))
    fw5 = es.enter_context(nc.sbuf_tensor("fw5", [128, 400], F32))
    fw6 = es.enter_context(nc.sbuf_tensor("fw6", [128, 400], F32))
    cbc = es.enter_context(nc.sbuf_tensor("cbc", [128, 128], F32))
    kbc = es.enter_context(nc.sbuf_tensor("kbc", [128, 128], F32))
    eqt = es.enter_context(nc.sbuf_tensor("eqt", [128, 128], F32))
    junkm = es.enter_context(nc.sbuf_tensor("junkm", [128, 128], F32))
    partials = es.enter_context(nc.sbuf_tensor("partials", [128, 8], F32))
    gb = es.enter_context(nc.sbuf_tensor("gb", [128, 4], F32))
    gc = es.enter_context(nc.sbuf_tensor("gc", [128, NCLS], F32))
    gcp = es.enter_context(nc.sbuf_tensor("gcp", [128, NCLS], F32))
    junk43 = es.enter_context(nc.sbuf_tensor("junk43", [128, NCLS], F32))
    junk4 = es.enter_context(nc.sbuf_tensor("junk4", [128, 4], F32))
    tbox = es.enter_context(nc.sbuf_tensor("tbox", [128, 4], F32))
    pvec = es.enter_context(nc.sbuf_tensor("pvec", [1, 8], F32))
    pp0 = es.enter_context(nc.psum_tensor("pp0", [128, 2048], F32))
    pp1 = es.enter_context(nc.psum_tensor("pp1", [128, 2048], F32))
    with es:
        pp = [pp0, pp1]
        psp = pp0[0:1, 0:8]

        names = [
            "sumx", "sumy", "csx", "csy", "gxf", "gyf", "bw", "bh", "bbs",
            "vbw", "vbh", "vbs", "vlab", "valid", "rq", "rm", "rf", "r2",
            "rr", "gxc", "gyc", "gx2", "gy2", "g2s", "w3a", "pen",
            "t2a", "t2b", "cellf", "cellgf", "labcf", "keyf",
            "cva", "sent", "kept", "keep2", "later1", "later2",
            "l1r", "negrow", "plab", "lnp", "ln1mp", "psq", "mpsq", "sumy2",
        ]
        col = {n: sc[:, i: i + 1] for i, n in enumerate(names)}

        idxp1 = pka[:, PK_CV + 0: PK_CV + 1]
        ones = pka[:, PK_CV + 1: PK_CV + 2]
        basef = pka[:, PK_CV + 2: PK_CV + 3]
        nc.const_aps.aps[(F32, 0.0)] = pka[:, PK_CV + 3: PK_CV + 4]
        nc.const_aps.aps[(F32, 1.0)] = ones
        nc.const_aps.aps[(F32, 1e-6)] = pka[:, PK_CV + 4: PK_CV + 5]
        nc.const_aps.aps[(F32, -LN4)] = pka[:, PK_CV + 5: PK_CV + 6]
        labf = pka[:, PK_LAB: PK_LAB + 1]
        chm = pka[:, PK_CHM: PK_CHM + NCLS]
        utri = pkb[:, PKB_UT: PKB_UT + 128]
        pkhm = pkb[:, PKB_HM: PKB_HM + 400]
        cellg = sci[:, 2:3]

        with nc.Block() as block:

            @block.sync
            def _(sync):
                sync.dma_start(out=pka[:, :], in_=pka_d[:, :]).then_inc(dma_pk, 16)
                sync.dma_start(out=sQ2[:, :], in_=q2_d[:, 0:6400]).then_inc(dma_in, 16)
                sync.dma_start(out=blkW[:, :], in_=q2_d[:, 6400:6912]).then_inc(dma_in, 16)
                sync.dma_start(out=pkb[:, :], in_=pkb_d[:, :]).then_inc(dma_pb, 16)
                sync.wait_ge(tr2_s, 1)
                sync.dma_start(out=dbc[0:1, :], in_=tmpT2[4:5, :]).then_inc(dma2, 16)
                sync.dma_start(out=dbc[1:2, :], in_=tmpT2[5:6, :]).then_inc(dma2, 16)
                sync.wait_ge(dma2, 32)
                sync.dma_start(out=cbc[:, :], in_=dbc[0:1, :].to_broadcast([128, 128])).then_inc(dma2, 16)
                sync.dma_start(out=kbc[:, :], in_=dbc[1:2, :].to_broadcast([128, 128])).then_inc(dma2, 16)
                # partial-sum vector out (host combines across cores)
                sync.wait_ge(pv_s, 1)
                sync.dma_start(out=out_d[:, :], in_=pvec[:, :]).then_inc(d6, 16)
                nd6 = 16
                if debug:
                    for nm, t in [("d_partials", partials), ("d_sc", sc),
                                  ("d_hmL", hmL), ("d_W5", W5)]:
                        sync.dma_start(out=dbg[nm][:, :], in_=t[:, :]).then_inc(d6, 16)
                        nd6 += 16
                sync.wait_ge(d6, nd6)

            @block.scalar
            def _(scalar):
                scalar.wait_ge(va, 1)
                scalar.activation(col["rq"], col["bbs"], ACT.Ln)
                scalar.drain()
                scalar.activation(col["rq"], col["rq"], ACT.Exp, bias=-LN4, scale=0.5)
                scalar.activation(tbox[:, 2:4], sc[:, 6:8], ACT.Ln, bias=1e-6, scale=0.25)
                scalar.drain()
                scalar.sem_inc(av, 1)
                # early focal transcendentals from p (input-only)
                scalar.wait_ge(vf, 1)
                scalar.activation(fw2[:, :], fw6[:, :], ACT.Ln)
                scalar.activation(fw3[:, :], fw6[:, :], ACT.Ln, bias=1.0, scale=-1.0)
                scalar.activation(fw4[:, :], fw6[:, :], ACT.Square)
                scalar.activation(fw5[:, :], fw6[:, :], ACT.Square, bias=1.0, scale=-1.0)
                scalar.drain()
                scalar.sem_inc(av, 1)
                # cls sigmoid via exp
                scalar.wait_ge(g_s, 32)
                scalar.activation(gcp[:, :], gc[:, :], ACT.Exp, scale=-1.0)
                scalar.drain()
                scalar.sem_inc(av, 1)
                scalar.wait_ge(va, 2)
                scalar.activation(junk43[:, :], gcp[:, :], ACT.Ln, bias=1.0, scale=-1.0)
                scalar.activation(gc[:, :], gcp[:, :], ACT.Square)
                scalar.drain()
                scalar.sem_inc(av, 1)
                scalar.wait_ge(va, 3)
                scalar.activation(col["lnp"], col["plab"], ACT.Ln)
                scalar.activation(col["ln1mp"], col["plab"], ACT.Ln, bias=1.0, scale=-1.0)
                scalar.activation(col["psq"], col["plab"], ACT.Square)
                scalar.activation(col["mpsq"], col["plab"], ACT.Square, bias=1.0, scale=-1.0)
                scalar.drain()
                scalar.sem_inc(av, 1)
                # t = exp(clamped log heatmap)
                scalar.wait_ge(va, 4)
                scalar.activation(fw1[:, :], fw0[:, :], ACT.Exp)
                scalar.drain()
                scalar.sem_inc(av, 1)

            @block.tensor
            def _(tensor):
                tensor.wait_ge(wsem, 64)
                tensor.wait_ge(dma_in, 32)
                for g in range(NBANK):
                    q = min(g // 4, 12)
                    pt = pp[q % 2]
                    off = (g % 4) * 512 if g < 48 else (g - 48) * 512
                    if q >= 2 and g % 4 == 0 or g == 48:
                        tensor.wait_ge(dv_s, q - 1)
                    tensor.matmul(
                        pt[:, off: off + 512],
                        sQ2[:, g * 128: (g + 1) * 128],
                        blkW[:, :],
                        start=True,
                        stop=True,
                        skip_group_check=True,
                    ).then_inc(pe_s, 1)
                tensor.wait_ge(pt_s, 1)
                tensor.matmul(psp, ones, partials[:, :], start=True, stop=True, skip_group_check=True).then_inc(pp_s, 1)

            @block.gpsimd
            def _(gpsimd):
                gpsimd.wait_ge(dma_in, 32)
                gpsimd.wait_ge(tr_s, 1)
                for c in range(4):
                    gpsimd.dma_start(
                        out=blkW[4 * c: 4 * c + 4, c * 128: (c + 1) * 128],
                        in_=tmpT[0:4, :],
                    ).then_inc(wsem, 16)
                gpsimd.wait_ge(cell_s, 1)
                gpsimd.indirect_dma_start(
                    out=gb[:, :], out_offset=None,
                    in_=pb_d[:, :],
                    in_offset=bass.IndirectOffsetOnAxis(ap=cellg, axis=0),
                ).then_inc(g_s, 16)
                gpsimd.indirect_dma_start(
                    out=gc[:, :], out_offset=None,
                    in_=pc_d[:, :],
                    in_offset=bass.IndirectOffsetOnAxis(ap=cellg, axis=0),
                ).then_inc(g_s, 16)
                if USE_F16:
                    # fp16 cast staging of each PSUM pair (frees the bank for PE)
                    for gp in range(NPAIR):
                        gpsimd.wait_ge(pe_s, 2 * (gp + 1))
                        gpsimd.dma_start(
                            out=stg[:, (gp % PDEPTH) * 1024: (gp % PDEPTH) * 1024 + 1024],
                            in_=pp[gp % PDEPTH][:, :],
                        ).then_inc(st_s, 16)

            @block.vector
            def _(v):
                ts, stt = v.tensor_scalar, v.scalar_tensor_tensor

                def D():
                    v.drain()

                v.wait_ge(dma_pk, 16)
                # ---- stage A (area first: scalar round-trip starts asap) ----
                v.tensor_sub(sc[:, 6:8], pka[:, 2:4], pka[:, 0:2])
                D()
                v.tensor_mul(col["bbs"], col["bw"], col["bh"])
                D()
                v.sem_inc(va, 1)
                v.tensor_add(sc[:, 0:2], pka[:, 0:2], pka[:, 2:4])
                D()
                ts(sc[:, 2:4], sc[:, 0:2], 0.125, 0.5, op0=ALU.mult, op1=ALU.subtract)
                D()
                v.tensor_copy(sci[:, 0:2], sc[:, 2:4])
                D()
                ts(sc[:, 4:6], sci[:, 0:2], 0.0, 159.0, op0=ALU.max, op1=ALU.min)
                D()
                v.tensor_reduce(out=col["rm"], in_=pka[:, 0:4], op=ALU.add, axis=mybir.AxisListType.X)
                D()
                ts(sc[:, 9:11], sc[:, 6:8], 0.0, None, op0=ALU.is_gt)
                D()
                ts(col["vbs"], col["rm"], 0.0, None, op0=ALU.is_gt)
                D()
                ts(col["vlab"], labf, 0.0, None, op0=ALU.is_ge)
                D()
                v.tensor_reduce(out=col["valid"], in_=sc[:, 9:13], op=ALU.mult, axis=mybir.AxisListType.X)
                D()
                ts(sc[:, 19:21], sc[:, 4:6], CTR, None, op0=ALU.subtract)
                D()
                v.tensor_mul(sc[:, 21:23], sc[:, 19:21], sc[:, 19:21])
                D()
                v.tensor_add(col["g2s"], col["gx2"], col["gy2"])
                D()
                ts(col["pen"], col["valid"], 1.0, 1e30, op0=ALU.subtract, op1=ALU.mult)
                D()
                v.wait_ge(av, 1)
                ts(col["rm"], col["rq"], 2.0, 0.5, op0=ALU.max, op1=ALU.subtract)
                D()
                v.tensor_copy(sci[:, 3:4], col["rm"])
                D()
                v.tensor_copy(col["rf"], sci[:, 3:4])
                D()
                v.tensor_mul(col["r2"], col["rf"], col["rf"])
                D()
                v.reciprocal(col["rr"], col["r2"])
                D()
                ts(W5[:, 0:1], col["rr"], -2.0, None, op0=ALU.mult)
                D()
                v.tensor_mul(col["w3a"], W5[:, 0:1], col["g2s"])
                D()
                v.tensor_add(W5[:, 3:4], col["w3a"], col["pen"])
                D()
                ts(sc[:, 26:28], sc[:, 19:21], W5[:, 0:1], None, op0=ALU.mult)
                D()
                ts(W5[:, 1:3], sc[:, 26:28], -2.0, None, op0=ALU.mult)
                D()
                for c4 in range(4):
                    v.transpose(tmpT[0:32, c4 * 32: (c4 + 1) * 32], W5[c4 * 32: (c4 + 1) * 32, 0:32])
                D()
                v.sem_inc(tr_s, 1)
                # cell/key
                stt(col["cellf"], col["gyf"], 160.0, col["gxf"], op0=ALU.mult, op1=ALU.add)
                D()
                v.tensor_add(col["cellgf"], col["cellf"], basef)
                D()
                v.tensor_copy(cellg, col["cellgf"])
                D()
                v.sem_inc(cell_s, 1)
                ts(col["labcf"], labf, 0.0, 42.0, op0=ALU.max, op1=ALU.min)
                D()
                stt(col["keyf"], col["cellgf"], 43.0, col["labcf"], op0=ALU.mult, op1=ALU.add)
                D()
                v.tensor_mul(col["cva"], col["cellgf"], col["valid"])
                D()
                stt(col["sent"], col["valid"], 1.0, idxp1, op0=ALU.subtract, op1=ALU.mult)
                D()
                v.tensor_add(W5[:, 4:5], col["cva"], col["sent"])
                D()
                v.tensor_mul(col["cva"], col["keyf"], col["valid"])
                D()
                v.tensor_add(W5[:, 5:6], col["cva"], col["sent"])
                D()
                for c4 in range(4):
                    v.transpose(tmpT2[0:32, c4 * 32: (c4 + 1) * 32], W5[c4 * 32: (c4 + 1) * 32, 0:32])
                D()
                v.sem_inc(tr2_s, 1)
                v.tensor_sub(tbox[:, 0:2], sc[:, 2:4], sc[:, 4:6])
                D()
                # ---- focal prelude from input heatmap ----
                v.wait_ge(dma_pb, 16)
                ts(fw6[:, :], pkhm, EPS, 1.0 - EPS, op0=ALU.max, op1=ALU.min)  # p
                D()
                v.sem_inc(vf, 1)      # scalar: fw2..fw5
                # ---- dedup ----
                v.wait_ge(dma2, 64)
                ts(eqt[:, :], cbc[:, :], W5[:, 4:5], None, op0=ALU.is_equal)
                D()
                v.tensor_mul(junkm[:, :], eqt[:, :], utri)
                D()
                v.tensor_reduce(out=col["later1"], in_=junkm[:, :], op=ALU.max, axis=mybir.AxisListType.X)
                D()
                ts(col["cva"], col["later1"], -1.0, 1.0, op0=ALU.mult, op1=ALU.add)
                D()
                v.tensor_mul(partials[:, 2:3], col["valid"], col["cva"])
                D()
                v.tensor_copy(col["kept"], partials[:, 2:3])
                D()
                ts(eqt[:, :], kbc[:, :], W5[:, 5:6], None, op0=ALU.is_equal)
                D()
                v.tensor_mul(junkm[:, :], eqt[:, :], utri)
                D()
                v.tensor_reduce(out=col["later2"], in_=junkm[:, :], op=ALU.max, axis=mybir.AxisListType.X)
                D()
                ts(col["cva"], col["later2"], -1.0, 1.0, op0=ALU.mult, op1=ALU.add)
                D()
                v.tensor_mul(partials[:, 5:6], col["valid"], col["cva"])
                D()
                v.tensor_copy(col["keep2"], partials[:, 5:6])
                D()

                def reduce_quad(q):
                    if q < 12:
                        v.wait_ge(pe_s, 4 * (q + 1))
                        src = pp[q % 2][:, :]
                        na, lo = 16, 16 * q
                    else:
                        v.wait_ge(pe_s, 50)
                        src = pp[0][:, 0:1024]
                        na, lo = 8, 192
                    v.tensor_reduce(
                        out=hmL[:, :].rearrange("p (i f) -> p f i", i=2)[:, lo: lo + na, :],
                        in_=src.rearrange("p (a b m) -> p a b m", a=na, b=2, m=64),
                        op=ALU.max,
                        axis=mybir.AxisListType.X,
                    )
                    v.drain().then_inc(dv_s, 1)

                nxt = 0
                reduce_quad(nxt); nxt += 1
                # ---- box l1 ----
                v.wait_ge(g_s, 32)
                v.tensor_sub(junk4[:, :], gb[:, :], tbox[:, :])
                D()
                ts(gb[:, :], junk4[:, :], -1.0, None, op0=ALU.mult)
                D()
                v.tensor_tensor(junk4[:, :], junk4[:, :], gb[:, :], op=ALU.max)
                D()
                v.tensor_reduce(out=col["l1r"], in_=junk4[:, :], op=ALU.add, axis=mybir.AxisListType.X)
                D()
                v.tensor_mul(partials[:, 3:4], col["l1r"], col["kept"])
                D()
                reduce_quad(nxt); nxt += 1
                # ---- cls neg ----
                v.wait_ge(av, 3)
                ts(gcp[:, :], gcp[:, :], 1.0, None, op0=ALU.add)
                D()
                v.reciprocal(gcp[:, :], gcp[:, :])
                D()
                ts(gcp[:, :], gcp[:, :], EPS, 1.0 - EPS, op0=ALU.max, op1=ALU.min)
                D()
                v.sem_inc(va, 1)
                reduce_quad(nxt); nxt += 1
                v.wait_ge(av, 4)
                stt(junk43[:, :], gc[:, :], -0.75, junk43[:, :], op0=ALU.mult, op1=ALU.mult, accum_out=col["negrow"])
                D()
                v.tensor_mul(partials[:, 4:5], col["negrow"], col["kept"])
                D()
                # ---- cls pos ----
                ts(eqt[:, 0:NCLS], chm, col["labcf"], None, op0=ALU.is_equal)
                D()
                v.tensor_mul(junk43[:, :], gcp[:, :], eqt[:, 0:NCLS])
                D()
                v.tensor_reduce(out=col["plab"], in_=junk43[:, :], op=ALU.add, axis=mybir.AxisListType.X)
                D()
                v.sem_inc(va, 1)
                reduce_quad(nxt); nxt += 1
                v.wait_ge(av, 5)
                stt(col["cva"], col["mpsq"], -0.25, col["lnp"], op0=ALU.mult, op1=ALU.mult)
                D()
                stt(col["sent"], col["psq"], -0.75, col["ln1mp"], op0=ALU.mult, op1=ALU.mult)
                D()
                v.tensor_sub(col["cva"], col["cva"], col["sent"])
                D()
                v.tensor_mul(partials[:, 6:7], col["cva"], col["keep2"])
                D()
                v.memset(partials[:, 7:8], 0.0)
                D()
                # focal planes A (fw2) and B' (fw3) — input-only, hide before reduces
                v.wait_ge(av, 2)
                stt(fw2[:, :], fw5[:, :], -0.25, fw2[:, :], op0=ALU.mult, op1=ALU.mult)  # A
                D()
                stt(fw3[:, :], fw4[:, :], 0.75, fw3[:, :], op0=ALU.mult, op1=ALU.mult)   # B'
                D()
                # ---- remaining reduce quads ----
                while nxt < 13:
                    reduce_quad(nxt); nxt += 1
                # ---- heat focal tail ----
                ts(fw0[:, :], hmL[:, :], -80.0, None, op0=ALU.max)
                D()
                v.sem_inc(va, 1)                    # scalar: fw1 = exp(fw0)
                ts(fw4[:, :], fw0[:, :], THR, None, op0=ALU.is_ge)       # keep mask
                D()
                # pos in log domain: (fw0 > ln 0.5) * mask
                stt(fw5[:, :], fw0[:, :], LNH, fw4[:, :], op0=ALU.is_gt, op1=ALU.mult)
                D()
                v.tensor_reduce(out=partials[:, 0:1], in_=fw5[:, :], op=ALU.add, axis=mybir.AxisListType.X)
                D()
                v.tensor_mul(fw2[:, :], fw2[:, :], fw5[:, :])            # G1 = A*pos (during exp)
                D()
                v.wait_ge(av, 6)
                v.tensor_mul(fw1[:, :], fw1[:, :], fw4[:, :])            # t
                D()
                stt(fw4[:, :], fw1[:, :], 1.0, fw2[:, :], op0=ALU.mult, op1=ALU.mult,
                    accum_out=col["cva"])                                # t*G1, sum
                D()
                stt(fw6[:, :], fw1[:, :], 1.0, fw3[:, :], op0=ALU.subtract, op1=ALU.mult,
                    accum_out=col["sumy2"])                              # Y=(t-1)*B', sum(Y)
                D()
                stt(fw6[:, :], fw6[:, :], 1.0, fw5[:, :], op0=ALU.mult, op1=ALU.mult,
                    accum_out=col["sent"])                               # Y*pos, sum
                D()
                v.tensor_add(col["cva"], col["cva"], col["sumy2"])
                D()
                v.tensor_sub(partials[:, 1:2], col["cva"], col["sent"])
                D()
                v.sem_inc(pt_s, 1)
                v.wait_ge(pp_s, 1)
                v.tensor_copy(pvec[:, :], psp)
                D()
                v.sem_inc(pv_s, 1)

    return nc


_CACHE = {}


def _consts():
    p = np.arange(128)
    g = np.arange(NBANK)
    qg2 = np.zeros((16, 6912), np.float32)
    for c in range(4):
        pix = p[None, :] * 200 + 4 * g[:, None] + c
        xx = (pix % W).astype(np.float32) - CTR
        yy = (pix // W).astype(np.float32) - CTR
        q4 = np.stack([xx * xx + yy * yy, xx, yy, np.ones_like(xx)])
        qg2[4 * c: 4 * c + 4, 0:6400] = q4.reshape(4, 6400)
    utri = np.triu(np.ones((128, 128), dtype=np.float32), k=1)
    cvec = np.zeros((128, 8), dtype=np.float32)
    cvec[:, 0] = np.arange(128) + 1.0
    cvec[:, 1] = 1.0
    cvec[64:, 2] = PIX
    cvec[:, 4] = 1e-6
    cvec[:, 5] = -LN4
    chm = np.broadcast_to(np.arange(NCLS, dtype=np.float32), (128, NCLS))
    return qg2, utri, cvec, chm


def _pack(bb, lab32, hmf, utri, cvec, chm):
    pka = np.zeros((128, PKA_N), dtype=np.float32)
    pka[:, PK_BB: PK_BB + 4] = bb.reshape(128, 4)
    pka[:, PK_LAB] = lab32.reshape(128).astype(np.float32)
    pka[:, PK_CV: PK_CV + 8] = cvec
    pka[:, PK_CHM: PK_CHM + NCLS] = chm
    pkb = np.zeros((128, PKB_N), dtype=np.float32)
    pkb[:, PKB_UT: PKB_UT + 128] = utri
    pkb[:, PKB_HM: PKB_HM + 400] = hmf.reshape(BPC, 128, 200).transpose(1, 0, 2).reshape(128, 400)
    return pka, pkb


def _combine(pvecs):
    """Final cross-core reduction + divides, mirroring the reference math."""
    P = np.zeros(8, dtype=np.float32)
    for v in pvecs:
        P = P + v.astype(np.float32)
    heat = P[1] / max(P[0], np.float32(1.0))
    if P[2] > 1.0:
        box = P[3] / max(P[2], np.float32(1.0))
        cls = (P[4] + P[6]) / max(P[5], np.float32(1.0))
    else:
        box = np.float32(0.0)
        cls = np.float32(0.0)
    return np.float32(heat + box + cls)


def kernel(pred_heatmap, pred_boxes, pred_classes, bboxes, labels):
    if "nc" not in _CACHE:
        _CACHE["nc"] = _build()
    nc = _CACHE["nc"]

    qg2, utri, cvec, chm = _consts()
    pbt = np.ascontiguousarray(pred_boxes.transpose(0, 2, 3, 1).reshape(B, PIX, 4))
    pct = np.ascontiguousarray(pred_classes.transpose(0, 2, 3, 1).reshape(B, PIX, NCLS))
    hmf = np.ascontiguousarray(pred_heatmap.reshape(B, PIX)).astype(np.float32)
    lab32 = np.asarray(labels).astype(np.int32)

    in_maps = []
    for c in range(NC):
        s = slice(c * BPC, (c + 1) * BPC)
        pka, pkb = _pack(np.asarray(bboxes[s], dtype=np.float32), lab32[s], hmf[s], utri, cvec, chm)
        in_maps.append({
            "pka": pka, "pkb": pkb, "q2": qg2,
            "pbt": pbt[s].reshape(BPC * PIX, 4),
            "pct": pct[s].reshape(BPC * PIX, NCLS),
        })

    r = run_bass_kernel_spmd(nc, in_maps, list(range(NC)))
    return _combine([np.asarray(r.results[c]["out"]).reshape(8) for c in range(NC)])


if __name__ == "__main__":
    import reference
    inputs = reference.setup_inputs()
    inputs = {k: np.asarray(v) for k, v in inputs.items()}
    out = kernel(**inputs)
    exp = np.asarray(reference.reference(**{k: v for k, v in inputs.items()}))
    rel = abs(out - exp) / max(abs(exp), 1e-9)
    print("expected:", exp, "actual:", out, "rel:", rel)
